# revision 10
# baseline (speedup 1.0000x reference)
"""Distributed ContrastiveMoCoKnnBert loss kernel for 8 trn2 NeuronCores.

Math reduction (exact, not approximate):
  loss_con = -mean(log_softmax([pos | negs] / T)[:, 0]) over (B*TOP_K) rows.
  For row (b, j):  term = log(exp(p_bj/T) + sum_neg exp(n/T)) - p_bj/T
  where p_bj = j-th largest of cos_sim[b, :] (over ALL K columns) and the
  negative sum runs over columns whose queue label != labels[b].  The
  reference's top-NEG_MIN sort is irrelevant: softmax denominators are
  permutation invariant.  So the kernel only needs, per batch row:
    * top-25 values of cos_sim[b, :]        (monotonic under exp -> we
      extract top exp-values instead)
    * S_all[b] = sum_k exp(cos/T), S_pos[b] = sum_{label match} exp(cos/T)

Sharding: feature_queue is sorted by label on the host (1024 rows per
label, exactly balanced by construction), transposed, tiled, and split
along K into 8 shards of 8192 (= 8 labels x 1024) -- one per core:
  1. linerT = (tanh(q@Wd+bd)@Wo+bo).T unnormalized via transpose-free
     matmuls (host supplies qT in partition-major layout; Wd/Wo/Wc1
     travel as fp8e4m3 x64 -- the x64 is folded back out in the
     activation scale), plus its column norms via a DVE square-sum and
     a ones-matmul partition reduction
  2. stream the fp8(e4m3, x256 host scale) fqT shard through TensorE:
     cos chunks [128, 512] f32 psum (partition = batch b + 64*half);
     the two 64-partition halves are emitted interleaved so they run
     concurrently in separate PE column groups (2x column tiling); two
     512-col chunks share one [128, 1024] psum pair-tile
  3. one ScalarE Exp per pair-tile with per-partition scale
     (2/(256*norm_b)) fused with the accumulate-sum -> acc[128, 4];
     each acc column IS one label-group sum (1024 sorted columns)
  4. one VectorE MAX8 per pair-tile -> top-8 bucket candidates
     cand[128, 32]; host PROVES completeness (bucket 8th-largest <=
     global 25th candidate) -- holds by a huge margin for random data
  5. classification head logits (replicated) -> lcT[63, 64]
Host merges: top-25 of the 512 per-row candidates, S_neg = S_all-S_pos,
and assembles the scalar loss in f64.  All O(B*K*H) work is on device.

DMA strategy: every input tensor is one DMA_DIRECT2D instruction on the
sync HWDGE queue, in critical-path order (head weights before the fq
stream) -- the sync engine pays ~620ns of descriptor generation per
instruction and the queue is FIFO, so order == priority.  All fq tiles
prefetch into SBUF; the kernel end-to-end is HBM-bandwidth-bound.
"""

import os

import numpy as np

import concourse.bass as bass
import concourse.bacc as bacc
import concourse.tile as tile
from concourse import mybir
from concourse.bass_utils import run_bass_kernel_spmd

B = 64
H = 768
K = 65536
L = 64            # NUM_LABELS
TOP_K = 25
T = 0.5
NCORES = 8
KSH = K // NCORES         # 8192 queue rows per core
NKC = H // 128            # 6 contraction chunks
CHUNK = 512               # psum-bank sized cos chunk
NJ = KSH // 2 // CHUNK    # 8 chunks per half
NP = NJ // 2              # pair-tiles (= label groups per half)
NH = 2                    # halves (partition packing: p = b + 64*h)
NCAND = 8                 # top-8 extracted per 1024-col pair-tile

F32 = mybir.dt.float32
BF16 = mybir.dt.bfloat16
FP8 = mybir.dt.float8e4
FQ_SCALE = 256.0          # host-side fp8 scale on the feature queue
W_SCALE = 64.0            # host-side fp8 scale on the head weights

_cache: dict = {}

last_exec_time_ns: int | None = None
last_results = None


def _ensure_ntff_hook():
    """Register the axon NTFF profiling hook if the image's antenv lacks
    the ``axon_hooks`` module (the hook impl itself ships in
    trn_agent_boot).  Also keep trace artifacts local instead of
    uploading to a share bucket."""
    import sys
    import types

    import concourse.bass_utils as bu

    bu.upload_artifacts = lambda tmpdir: tmpdir
    try:
        from antenv.axon_hooks import get_axon_ntff_profile_hook  # noqa: F401
        return
    except ImportError:
        pass
    try:
        from trn_agent_boot.trn_boot import _ntff_profile_via_ctypes
    except ImportError:
        return
    mod = types.ModuleType("antenv.axon_hooks")
    _hook = [None]
    mod.set_axon_ntff_profile_hook = lambda h: _hook.__setitem__(0, h)
    mod.get_axon_ntff_profile_hook = lambda: _hook[0]
    sys.modules["antenv.axon_hooks"] = mod
    import antenv

    antenv.axon_hooks = mod
    try:
        mod.set_axon_ntff_profile_hook(
            _ntff_profile_via_ctypes("/opt/axon/libaxon_pjrt.so")
        )
    except Exception:
        mod.set_axon_ntff_profile_hook(None)


def _build_nc():
    nc = bacc.Bacc(
        "TRN2",
        target_bir_lowering=False,
        debug=False,
        enable_asserts=False,
        num_devices=NCORES,
    )

    qT = nc.dram_tensor("qT", [128, NKC, B], BF16, kind="ExternalInput")
    # wd | wo | wc1 packed into one fp8 tensor: a single 13.8KB-per-row DMA
    wpk = nc.dram_tensor("wpk", [128, 3, NKC, H], FP8, kind="ExternalInput")
    wc2 = nc.dram_tensor("wc2", [128, NKC, L - 1], BF16, kind="ExternalInput")
    bpack = nc.dram_tensor("bpack", [128, 3 * NKC], F32, kind="ExternalInput")
    bc2t = nc.dram_tensor("bc2", [L - 1, 1], F32, kind="ExternalInput")
    # fq tiles pairwise row-packed: 12KB rows for pair DMAs; the last pair
    # is fetched as two singles so the serial tail works on 512 columns
    fqt = nc.dram_tensor(
        "fqt", [NP, 128, 2, NKC * NH * CHUNK], FP8, kind="ExternalInput"
    )

    # cand: [3 pairs * 8 | j6 * 8 | j7 * 8]; acc: [3 pairs | j6 | j7]
    cand_o = nc.dram_tensor("cand", [128, (NP + 1) * NCAND], BF16, kind="ExternalOutput")
    acc_o = nc.dram_tensor("acc", [128, NP + 2], F32, kind="ExternalOutput")
    lc_o = nc.dram_tensor("lcT", [L - 1, B], F32, kind="ExternalOutput")

    AF = mybir.ActivationFunctionType
    ALU = mybir.AluOpType

    with tile.TileContext(nc) as tc:
        with (
            tc.tile_pool(name="weights", bufs=1) as wpool,
            tc.tile_pool(name="work", bufs=1) as spool,
            tc.tile_pool(name="fqstream", bufs=NP) as fqpool,
            tc.tile_pool(name="expbuf", bufs=2) as epool,
            tc.tile_pool(name="cospair", bufs=2, space="PSUM") as pspool,
            tc.tile_pool(name="cossing", bufs=2, space="PSUM") as sspool,
            tc.tile_pool(name="headpsum", bufs=2, space="PSUM") as hpool,
        ):
            # ---- resident params --------------------------------------
            wpk_sb = wpool.tile([128, 3, NKC, H], FP8)
            wd_sb = wpk_sb[:, 0]
            wo_sb = wpk_sb[:, 1]
            wc1_sb = wpk_sb[:, 2]
            wc2_sb = wpool.tile([128, NKC, L - 1], BF16)
            qt_sb = wpool.tile([128, NKC, B], BF16)
            bias_sb = wpool.tile([128, 3 * NKC], F32)
            bc2_sb = wpool.tile([L - 1, 1], F32)
            ones_sb = wpool.tile([128, 1], F32)
            dummy_sb = wpool.tile([128, 1], F32)

            bd_sb = bias_sb[:, 0:NKC]
            bo_sb = bias_sb[:, NKC:2 * NKC]
            bc1_sb = bias_sb[:, 2 * NKC:3 * NKC]

            # warm the scalar engine's activation table (tanh/exp share a
            # table) while the weight DMAs are in flight
            nc.vector.memset(dummy_sb[:], 0.0)
            nc.scalar.activation(dummy_sb[:], dummy_sb[:], AF.Tanh)
            nc.vector.memset(ones_sb[:], 1.0)

            # one DMA instruction per tensor, in critical-path order, all
            # on the same FIFO queue; the last fq pair travels as two
            # singles so the tail computes on 512 columns
            nc.sync.dma_start(qt_sb[:], qT.ap())
            nc.sync.dma_start(bias_sb[:], bpack.ap())
            nc.sync.dma_start(wpk_sb[:], wpk.ap())
            nc.sync.dma_start(wc2_sb[:], wc2.ap())
            nc.sync.dma_start(bc2_sb[:], bc2t.ap())
            fts = []
            for jp in range(NP):
                ft = fqpool.tile([128, 2, NKC, NH * CHUNK], FP8, tag="fq")
                if jp < NP - 1:
                    nc.sync.dma_start(ft[:], fqt.ap()[jp])
                else:
                    nc.sync.dma_start(ft[:, 0], fqt.ap()[jp][:, 0])
                    nc.sync.dma_start(ft[:, 1], fqt.ap()[jp][:, 1])
                fts.append(ft)

            # ---- contrastive head: linerT (unnormalized) + norms ------
            h1_sb = spool.tile([128, NKC, B], BF16)
            pre2_sb = spool.tile([128, NKC, B], F32)
            pre2b_sb = spool.tile([128, NKC, B], BF16)
            sq_sb = spool.tile([128, NKC, B], F32)

            for mc in range(NKC):
                ps = hpool.tile([128, B], F32, tag="headps")
                for kc in range(NKC):
                    nc.tensor.matmul(
                        ps[:],
                        wd_sb[:, kc, mc * 128:(mc + 1) * 128],
                        qt_sb[:, kc, :],
                        start=(kc == 0),
                        stop=(kc == NKC - 1),
                    )
                nc.scalar.activation(
                    h1_sb[:, mc, :], ps[:], AF.Tanh,
                    bias=bd_sb[:, mc:mc + 1], scale=1.0 / W_SCALE,
                )

            for mc in range(NKC):
                ps = hpool.tile([128, B], F32, tag="headps")
                for kc in range(NKC):
                    nc.tensor.matmul(
                        ps[:],
                        wo_sb[:, kc, mc * 128:(mc + 1) * 128],
                        h1_sb[:, kc, :],
                        start=(kc == 0),
                        stop=(kc == NKC - 1),
                    )
                nc.vector.tensor_scalar(
                    pre2_sb[:, mc, :], ps[:],
                    1.0 / W_SCALE, bo_sb[:, mc:mc + 1],
                    ALU.mult, ALU.add,
                )
                nc.vector.tensor_copy(pre2b_sb[:, mc, :], pre2_sb[:, mc, :])
                nc.vector.tensor_mul(sq_sb[:, mc, :], pre2_sb[:, mc, :], pre2_sb[:, mc, :])

            # row norms of liner_q: DVE-accumulate the squares over the mc
            # slices, then one ones-matmul per partition half
            sqs_sb = spool.tile([128, B], F32)
            nc.vector.tensor_add(sqs_sb[:], sq_sb[:, 0, :], sq_sb[:, 1, :])
            for mc in range(2, NKC):
                nc.vector.tensor_add(sqs_sb[:], sqs_sb[:], sq_sb[:, mc, :])
            ps_n = hpool.tile([128, 1], F32, tag="headps")
            for hh in range(NH):
                nc.tensor.matmul(
                    ps_n[hh * 64:(hh + 1) * 64, :],
                    sqs_sb[:],
                    ones_sb[:],
                    start=True,
                    stop=True,
                )

            # ---- classification head (replicated, tiny) ---------------
            h1c_sb = spool.tile([128, NKC, B], BF16)
            for mc in range(NKC):
                ps = hpool.tile([128, B], F32, tag="headps")
                for kc in range(NKC):
                    nc.tensor.matmul(
                        ps[:],
                        wc1_sb[:, kc, mc * 128:(mc + 1) * 128],
                        qt_sb[:, kc, :],
                        start=(kc == 0),
                        stop=(kc == NKC - 1),
                    )
                nc.scalar.activation(
                    h1c_sb[:, mc, :], ps[:], AF.Tanh,
                    bias=bc1_sb[:, mc:mc + 1], scale=1.0 / W_SCALE,
                )

            norm_sb = spool.tile([128, 1], F32)
            rcp_sb = spool.tile([128, 1], F32)
            scol_sb = spool.tile([128, 1], F32)
            nc.scalar.activation(norm_sb[:], ps_n[:], AF.Sqrt)
            nc.vector.reciprocal(rcp_sb[:], norm_sb[:])
            nc.vector.tensor_scalar_mul(scol_sb[:], rcp_sb[:], 1.0 / (T * FQ_SCALE))

            ps_l = hpool.tile([L - 1, B], F32, tag="headps")
            for kc in range(NKC):
                nc.tensor.matmul(
                    ps_l[:],
                    wc2_sb[:, kc, :],
                    h1c_sb[:, kc, :],
                    start=(kc == 0),
                    stop=(kc == NKC - 1),
                )
            lc_sb = spool.tile([L - 1, B], F32)
            nc.vector.tensor_scalar_add(lc_sb[:], ps_l[:], bc2_sb[:])
            nc.sync.dma_start(lc_o.ap(), lc_sb[:])

            # ---- main stream: cos pair-tiles -> exp(+sum) -> topk -----
            # two 512-col chunks accumulate into one [128, 1024] psum
            # tile; the two partition halves alternate so the PE runs
            # them concurrently in column groups (0,0) and (0,64).  The
            # last pair is processed as two 512-col steps to shorten the
            # serial exp/max8 tail after its DMA lands.
            acc_sb = spool.tile([128, NP + 2], F32)
            cand_sb = spool.tile([128, (NP + 1) * NCAND], BF16)

            for jp in range(NP - 1):
                ps_c = pspool.tile([128, NH * CHUNK], F32, tag="cos")
                ft = fts[jp]
                for half in range(2):
                    for kc in range(NKC):
                        for hh in range(NH):
                            nc.tensor.matmul(
                                ps_c[hh * 64:(hh + 1) * 64,
                                     half * CHUNK:(half + 1) * CHUNK],
                                pre2b_sb[:, kc, :],
                                ft[:, half, kc, hh * CHUNK:(hh + 1) * CHUNK],
                                start=(kc == 0),
                                stop=(kc == NKC - 1),
                            )
                exp_t = epool.tile([128, NH * CHUNK], BF16, tag="exp")
                nc.scalar.activation(
                    exp_t[:],
                    ps_c[:],
                    AF.Exp,
                    scale=scol_sb[:],
                    accum_out=acc_sb[:, jp:jp + 1],
                )
                nc.vector.max(cand_sb[:, jp * NCAND:(jp + 1) * NCAND], exp_t[:])

            for half in range(2):
                ps_c = sspool.tile([128, CHUNK], F32, tag="cos1")
                ft = fts[NP - 1]
                for kc in range(NKC):
                    for hh in range(NH):
                        nc.tensor.matmul(
                            ps_c[hh * 64:(hh + 1) * 64, :],
                            pre2b_sb[:, kc, :],
                            ft[:, half, kc, hh * CHUNK:(hh + 1) * CHUNK],
                            start=(kc == 0),
                            stop=(kc == NKC - 1),
                        )
                exp_t = epool.tile([128, CHUNK], BF16, tag="exp1")
                col = NP - 1 + half
                nc.scalar.activation(
                    exp_t[:],
                    ps_c[:],
                    AF.Exp,
                    scale=scol_sb[:],
                    accum_out=acc_sb[:, col:col + 1],
                )
                nc.vector.max(
                    cand_sb[:, (NP - 1 + half) * NCAND:(NP + half) * NCAND],
                    exp_t[:],
                )

            nc.sync.dma_start(cand_o.ap(), cand_sb[:])
            nc.gpsimd.dma_start(acc_o.ap(), acc_sb[:])

    nc.compile()
    return nc


def _get_nc():
    if "nc" not in _cache:
        _cache["nc"] = _build_nc()
    return _cache["nc"]


def _prep_inputs(q, label_queue, feature_queue, Wd, bd, Wo, bo, Wc1, bc1, Wc2, bc2):
    """Host-side shard/layout prep.  Returns per-core input maps."""
    lq = np.asarray(label_queue).astype(np.int64)
    counts = np.bincount(lq, minlength=L)
    assert counts.shape[0] == L and np.all(counts == K // L), (
        "kernel assumes an exactly balanced label queue"
    )
    perm = np.argsort(lq, kind="stable")
    fq_sorted = np.asarray(feature_queue, dtype=np.float32)[perm]  # [K, H]

    bf16 = mybir.dt.np(BF16)
    fp8 = mybir.dt.np(FP8)

    def pk(w, cols, dt, scale=1.0):  # [H, cols] -> [128, NKC, cols]
        return np.ascontiguousarray(
            (np.asarray(w, np.float32) * scale)
            .reshape(NKC, 128, cols).transpose(1, 0, 2)
        ).astype(dt)

    def pb(b):  # [H] -> [128, NKC]
        return np.asarray(b, np.float32).reshape(NKC, 128).T

    bpack = np.ascontiguousarray(
        np.concatenate([pb(bd), pb(bo), pb(bc1)], axis=1).astype(np.float32)
    )
    common = {
        "qT": pk(np.asarray(q, np.float32).T, B, bf16),
        "wpk": np.ascontiguousarray(np.stack(
            [pk(Wd, H, fp8, W_SCALE), pk(Wo, H, fp8, W_SCALE),
             pk(Wc1, H, fp8, W_SCALE)], axis=1
        )),
        "wc2": pk(Wc2, L - 1, bf16),
        "bpack": bpack,
        "bc2": np.ascontiguousarray(np.asarray(bc2, np.float32).reshape(L - 1, 1)),
    }
    in_maps = []
    for c in range(NCORES):
        shard = fq_sorted[c * KSH:(c + 1) * KSH]          # [8192, H]
        fqT = np.ascontiguousarray(shard.T)               # [H, 8192]
        # fqt[jp, r, half, kc*1024 + hh*512 + cc]
        #   = fqT[kc*128 + r, hh*4096 + (2*jp+half)*512 + cc]
        tiles = np.ascontiguousarray(
            (fqT * FQ_SCALE).reshape(NKC, 128, NH, NP, 2, CHUNK)
            .transpose(3, 1, 4, 0, 2, 5)
            .reshape(NP, 128, 2, NKC * NH * CHUNK).astype(fp8)
        )
        in_maps.append({**common, "fqt": tiles})
    return in_maps


def kernel(
    q,
    labels,
    label_queue,
    feature_queue,
    Wd,
    bd,
    Wo,
    bo,
    Wc1,
    bc1,
    Wc2,
    bc2,
):
    global last_exec_time_ns, last_results
    nc = _get_nc()
    in_maps = _prep_inputs(
        q, label_queue, feature_queue, Wd, bd, Wo, bo, Wc1, bc1, Wc2, bc2
    )

    trace = os.environ.get("BASS_KERNEL_TRACE", "0") == "1"
    if trace:
        _ensure_ntff_hook()
    try:
        res = run_bass_kernel_spmd(
            nc,
            in_maps,
            core_ids=list(range(NCORES)),
            trace=trace,
            trace_cores=[0] if trace else None,
        )
    except Exception:
        if not trace:
            raise
        res = run_bass_kernel_spmd(nc, in_maps, core_ids=list(range(NCORES)))
    last_exec_time_ns = res.exec_time_ns
    last_results = res

    labels_np = np.asarray(labels).astype(np.int64)

    # ---- tiny host-side merge (the "gather + reduce" step) -----------
    C = np.stack([np.asarray(r["cand"]) for r in res.results]).astype(np.float64)
    A = np.stack([np.asarray(r["acc"]) for r in res.results]).astype(np.float64)

    # per-row candidate pool: cores x halves x (5 buckets * top-8)
    cand = np.concatenate([C[:, :B, :], C[:, B:, :]], axis=2)  # [8, 64, 80]
    cand = cand.transpose(1, 0, 2).reshape(B, -1)              # [64, 640]
    e_top = np.sort(cand, axis=1)[:, ::-1][:, :TOP_K]          # exp(p/T) desc
    # Exactness proof: every unextracted value in a bucket is <= that
    # bucket's 8th-largest (MAX8 output is sorted desc).  If all bucket
    # minima are <= the global 25th candidate, the top-25 value set is
    # provably complete.
    bucket_min = np.concatenate(
        [C[:, :B, 7::8], C[:, B:, 7::8]], axis=2
    ).transpose(1, 0, 2).reshape(B, -1)                        # [64, 80]
    assert (bucket_min.max(axis=1) <= e_top[:, TOP_K - 1] + 1e-12).all(), (
        "top-k candidate extraction cannot prove exactness for this input"
    )

    # acc columns: [pair0 | pair1 | pair2 | j6 | j7]; label group g of
    # half hh is pair g for g < 3, else j6 + j7 (label = core*8 + hh*4+g,
    # partition p = b + 64*hh)
    S_all = A[:, :B, :].sum(axis=(0, 2)) + A[:, B:, :].sum(axis=(0, 2))  # [64]
    lam = labels_np
    c_star, r_star = np.divmod(lam, 8)
    h_star, g_star = np.divmod(r_star, 4)
    row = np.arange(B) + 64 * h_star
    S_pos = np.where(
        g_star < 3,
        A[c_star, row, np.minimum(g_star, 2)],
        A[c_star, row, 3] + A[c_star, row, 4],
    )
    S_neg = S_all - S_pos

    loss_con = float(np.mean(np.log(e_top + S_neg[:, None]) - np.log(e_top)))

    logits = np.asarray(res.results[0]["lcT"]).astype(np.float64).T  # [64, 63]
    m = logits.max(axis=1, keepdims=True)
    lse = np.log(np.exp(logits - m).sum(axis=1, keepdims=True)) + m
    logp = logits - lse
    loss_cls = float(-np.mean(logp[np.arange(B), labels_np]))

    loss = 0.5 * loss_con + 0.5 * loss_cls
    return np.asarray(loss, dtype=np.float32)


# revision 15
# speedup vs baseline: 1.0638x; 1.0638x over previous
"""Distributed ContrastiveMoCoKnnBert loss kernel for 8 trn2 NeuronCores.

Math reduction (exact, not approximate):
  loss_con = -mean(log_softmax([pos | negs] / T)[:, 0]) over (B*TOP_K) rows.
  For row (b, j):  term = log(exp(p_bj/T) + sum_neg exp(n/T)) - p_bj/T
  where p_bj = j-th largest of cos_sim[b, :] (over ALL K columns) and the
  negative sum runs over columns whose queue label != labels[b].  The
  reference's top-NEG_MIN sort is irrelevant: softmax denominators are
  permutation invariant.  So the kernel only needs, per batch row:
    * top-25 values of cos_sim[b, :]        (monotonic under exp -> we
      extract top exp-values instead)
    * S_all[b] = sum_k exp(cos/T), S_pos[b] = sum_{label match} exp(cos/T)

Sharding: feature_queue is sorted by label on the host (1024 rows per
label, exactly balanced by construction), transposed, tiled, and split
along K into 8 shards of 8192 (= 8 labels x 1024) -- one per core:
  1. linerT = (tanh(q@Wd+bd)@Wo+bo).T unnormalized via transpose-free
     matmuls (host supplies qT in partition-major layout; Wd/Wo/Wc1
     travel as fp8e4m3 x64 -- the x64 is folded back out in the
     activation scale), plus its column norms via a DVE square-sum and
     a ones-matmul partition reduction
  2. stream the fp8(e4m3, x256 host scale) fqT shard through TensorE:
     cos chunks [128, 512] f32 psum (partition = batch b + 64*half);
     the two 64-partition halves are emitted interleaved so they run
     concurrently in separate PE column groups (2x column tiling); two
     512-col chunks share one [128, 1024] psum pair-tile
  3. one ScalarE Exp per pair-tile with per-partition scale
     (2/(256*norm_b)) fused with the accumulate-sum -> acc[128, 4];
     each acc column IS one label-group sum (1024 sorted columns)
  4. one VectorE MAX8 per pair-tile -> top-8 bucket candidates
     cand[128, 32]; host PROVES completeness (bucket 8th-largest <=
     global 25th candidate) -- holds by a huge margin for random data
  5. classification head logits (replicated) -> lcT[63, 64]
Host merges: top-25 of the 512 per-row candidates, S_neg = S_all-S_pos,
and assembles the scalar loss in f64.  All O(B*K*H) work is on device.

DMA strategy: every input tensor is one DMA_DIRECT2D instruction on the
sync HWDGE queue, in critical-path order (head weights before the fq
stream) -- the sync engine pays ~620ns of descriptor generation per
instruction and the queue is FIFO, so order == priority.  All fq tiles
prefetch into SBUF; the kernel end-to-end is HBM-bandwidth-bound.
"""

import os

import numpy as np

import concourse.bass as bass
import concourse.bacc as bacc
import concourse.tile as tile
from concourse import mybir
from concourse.bass_utils import run_bass_kernel_spmd

B = 64
H = 768
K = 65536
L = 64            # NUM_LABELS
TOP_K = 25
T = 0.5
NCORES = 8
KSH = K // NCORES         # 8192 queue rows per core
NKC = H // 128            # 6 contraction chunks
CHUNK = 512               # psum-bank sized cos chunk
NJ = KSH // 2 // CHUNK    # 8 chunks per half
NP = NJ // 2              # pair-tiles (= label groups per half)
NH = 2                    # halves (partition packing: p = b + 64*h)
NCAND = 8                 # top-8 extracted per 1024-col pair-tile

F32 = mybir.dt.float32
BF16 = mybir.dt.bfloat16
FP8 = mybir.dt.float8e4
FQ_SCALE = 256.0          # host-side fp8 scale on the feature queue
W_SCALE = 64.0            # host-side fp8 scale on the head weights

_cache: dict = {}

last_exec_time_ns: int | None = None
last_results = None


def _ensure_ntff_hook():
    """Register the axon NTFF profiling hook if the image's antenv lacks
    the ``axon_hooks`` module (the hook impl itself ships in
    trn_agent_boot).  Also keep trace artifacts local instead of
    uploading to a share bucket."""
    import sys
    import types

    import concourse.bass_utils as bu

    bu.upload_artifacts = lambda tmpdir: tmpdir
    try:
        from antenv.axon_hooks import get_axon_ntff_profile_hook  # noqa: F401
        return
    except ImportError:
        pass
    try:
        from trn_agent_boot.trn_boot import _ntff_profile_via_ctypes
    except ImportError:
        return
    mod = types.ModuleType("antenv.axon_hooks")
    _hook = [None]
    mod.set_axon_ntff_profile_hook = lambda h: _hook.__setitem__(0, h)
    mod.get_axon_ntff_profile_hook = lambda: _hook[0]
    sys.modules["antenv.axon_hooks"] = mod
    import antenv

    antenv.axon_hooks = mod
    try:
        mod.set_axon_ntff_profile_hook(
            _ntff_profile_via_ctypes("/opt/axon/libaxon_pjrt.so")
        )
    except Exception:
        mod.set_axon_ntff_profile_hook(None)


def _build_nc():
    nc = bacc.Bacc(
        "TRN2",
        target_bir_lowering=False,
        debug=False,
        enable_asserts=False,
        num_devices=NCORES,
    )

    qT = nc.dram_tensor("qT", [128, NKC, B], BF16, kind="ExternalInput")
    wd = nc.dram_tensor("wd", [128, NKC, H], FP8, kind="ExternalInput")
    wo = nc.dram_tensor("wo", [128, NKC, H], FP8, kind="ExternalInput")
    wc1 = nc.dram_tensor("wc1", [128, NKC, H], FP8, kind="ExternalInput")
    wc2 = nc.dram_tensor("wc2", [128, NKC, L - 1], BF16, kind="ExternalInput")
    bpack = nc.dram_tensor("bpack", [128, 3 * NKC], F32, kind="ExternalInput")
    bc2t = nc.dram_tensor("bc2", [L - 1, 1], F32, kind="ExternalInput")
    # fq tiles pairwise row-packed: 12KB rows for pair DMAs; the last pair
    # is fetched as two singles so the serial tail works on 512 columns
    fqt = nc.dram_tensor(
        "fqt", [NP, 128, 2, NKC * NH * CHUNK], FP8, kind="ExternalInput"
    )

    # cand: [3 pairs * 8 | j6 * 8 | j7 * 8]; acc: [3 pairs | j6 | j7]
    cand_o = nc.dram_tensor("cand", [128, (NP + 1) * NCAND], BF16, kind="ExternalOutput")
    acc_o = nc.dram_tensor("acc", [128, NP + 2], F32, kind="ExternalOutput")
    lc_o = nc.dram_tensor("lcT", [L - 1, B], F32, kind="ExternalOutput")

    AF = mybir.ActivationFunctionType
    ALU = mybir.AluOpType

    with tile.TileContext(nc) as tc:
        with (
            tc.tile_pool(name="weights", bufs=1) as wpool,
            tc.tile_pool(name="work", bufs=1) as spool,
            tc.tile_pool(name="fqstream", bufs=NP) as fqpool,
            tc.tile_pool(name="expbuf", bufs=2) as epool,
            tc.tile_pool(name="cospair", bufs=2, space="PSUM") as pspool,
            tc.tile_pool(name="cossing", bufs=2, space="PSUM") as sspool,
            tc.tile_pool(name="headpsum", bufs=2, space="PSUM") as hpool,
        ):
            # ---- resident params --------------------------------------
            wd_sb = wpool.tile([128, NKC, H], FP8)
            wo_sb = wpool.tile([128, NKC, H], FP8)
            wc1_sb = wpool.tile([128, NKC, H], FP8)
            wc2_sb = wpool.tile([128, NKC, L - 1], BF16)
            qt_sb = wpool.tile([128, NKC, B], BF16)
            bias_sb = wpool.tile([128, 3 * NKC], F32)
            bc2_sb = wpool.tile([L - 1, 1], F32)
            ones_sb = wpool.tile([128, 1], F32)
            dummy_sb = wpool.tile([128, 1], F32)

            bd_sb = bias_sb[:, 0:NKC]
            bo_sb = bias_sb[:, NKC:2 * NKC]
            bc1_sb = bias_sb[:, 2 * NKC:3 * NKC]

            # warm the scalar engine's activation table (tanh/exp share a
            # table) while the weight DMAs are in flight
            nc.vector.memset(dummy_sb[:], 0.0)
            nc.scalar.activation(dummy_sb[:], dummy_sb[:], AF.Tanh)
            nc.vector.memset(ones_sb[:], 1.0)

            # one DMA instruction per tensor, in critical-path order, all
            # on the same FIFO queue; the last fq pair travels as two
            # singles so the tail computes on 512 columns
            nc.sync.dma_start(qt_sb[:], qT.ap())
            nc.sync.dma_start(bias_sb[:], bpack.ap())
            nc.sync.dma_start(wd_sb[:], wd.ap())
            nc.sync.dma_start(wo_sb[:], wo.ap())
            nc.sync.dma_start(wc1_sb[:], wc1.ap())
            nc.sync.dma_start(wc2_sb[:], wc2.ap())
            nc.sync.dma_start(bc2_sb[:], bc2t.ap())
            fts = []
            for jp in range(NP):
                ft = fqpool.tile([128, 2, NKC, NH * CHUNK], FP8, tag="fq")
                if jp < NP - 1:
                    nc.sync.dma_start(ft[:], fqt.ap()[jp])
                else:
                    nc.sync.dma_start(ft[:, 0], fqt.ap()[jp][:, 0])
                    nc.sync.dma_start(ft[:, 1], fqt.ap()[jp][:, 1])
                fts.append(ft)

            # ---- contrastive head: linerT (unnormalized) + norms ------
            h1_sb = spool.tile([128, NKC, B], BF16)
            pre2_sb = spool.tile([128, NKC, B], F32)
            pre2b_sb = spool.tile([128, NKC, B], BF16)
            sq_sb = spool.tile([128, NKC, B], F32)

            for mc in range(NKC):
                ps = hpool.tile([128, B], F32, tag="headps")
                for kc in range(NKC):
                    nc.tensor.matmul(
                        ps[:],
                        wd_sb[:, kc, mc * 128:(mc + 1) * 128],
                        qt_sb[:, kc, :],
                        start=(kc == 0),
                        stop=(kc == NKC - 1),
                    )
                nc.scalar.activation(
                    h1_sb[:, mc, :], ps[:], AF.Tanh,
                    bias=bd_sb[:, mc:mc + 1], scale=1.0 / W_SCALE,
                )

            for mc in range(NKC):
                ps = hpool.tile([128, B], F32, tag="headps")
                for kc in range(NKC):
                    nc.tensor.matmul(
                        ps[:],
                        wo_sb[:, kc, mc * 128:(mc + 1) * 128],
                        h1_sb[:, kc, :],
                        start=(kc == 0),
                        stop=(kc == NKC - 1),
                    )
                nc.vector.tensor_scalar(
                    pre2_sb[:, mc, :], ps[:],
                    1.0 / W_SCALE, bo_sb[:, mc:mc + 1],
                    ALU.mult, ALU.add,
                )
                nc.vector.tensor_copy(pre2b_sb[:, mc, :], pre2_sb[:, mc, :])
                nc.vector.tensor_mul(sq_sb[:, mc, :], pre2_sb[:, mc, :], pre2_sb[:, mc, :])

            # row norms of liner_q: DVE-accumulate the squares over the mc
            # slices, then one ones-matmul per partition half
            sqs_sb = spool.tile([128, B], F32)
            nc.vector.tensor_add(sqs_sb[:], sq_sb[:, 0, :], sq_sb[:, 1, :])
            for mc in range(2, NKC):
                nc.vector.tensor_add(sqs_sb[:], sqs_sb[:], sq_sb[:, mc, :])
            ps_n = hpool.tile([128, 1], F32, tag="headps")
            for hh in range(NH):
                nc.tensor.matmul(
                    ps_n[hh * 64:(hh + 1) * 64, :],
                    sqs_sb[:],
                    ones_sb[:],
                    start=True,
                    stop=True,
                )

            # scol = 1/(T * FQ_SCALE * ||liner_q||): the sqrt's two
            # activation-table swaps overlap the first pair's matmuls
            norm_sb = spool.tile([128, 1], F32)
            rcp_sb = spool.tile([128, 1], F32)
            scol_sb = spool.tile([128, 1], F32)
            nc.scalar.activation(norm_sb[:], ps_n[:], AF.Sqrt)
            nc.vector.reciprocal(rcp_sb[:], norm_sb[:])
            nc.vector.tensor_scalar_mul(scol_sb[:], rcp_sb[:], 1.0 / (T * FQ_SCALE))

            # ---- main stream: cos pair-tiles -> exp(+sum) -> topk -----
            # two 512-col chunks accumulate into one [128, 1024] psum
            # tile; the two partition halves alternate so the PE runs
            # them concurrently in column groups (0,0) and (0,64).  The
            # last pair is processed as two 512-col steps to shorten the
            # serial exp/max8 tail after its DMA lands.  The (tiny) cls
            # head is interleaved two mc-slices per pair so its matmuls
            # fill the PE's DMA-pacing gaps and its tanh chain never
            # blocks the stream (the logits matmuls run at the very end,
            # overlapping the exp/max8 tail).
            acc_sb = spool.tile([128, NP + 2], F32)
            cand_sb = spool.tile([128, (NP + 1) * NCAND], BF16)
            h1c_sb = spool.tile([128, NKC, B], BF16)

            def cls_slice(mc):
                ps = hpool.tile([128, B], F32, tag="headps")
                for kc in range(NKC):
                    nc.tensor.matmul(
                        ps[:],
                        wc1_sb[:, kc, mc * 128:(mc + 1) * 128],
                        qt_sb[:, kc, :],
                        start=(kc == 0),
                        stop=(kc == NKC - 1),
                    )
                nc.scalar.activation(
                    h1c_sb[:, mc, :], ps[:], AF.Tanh,
                    bias=bc1_sb[:, mc:mc + 1], scale=1.0 / W_SCALE,
                )

            for jp in range(NP - 1):
                ps_c = pspool.tile([128, NH * CHUNK], F32, tag="cos")
                ft = fts[jp]
                for half in range(2):
                    for kc in range(NKC):
                        for hh in range(NH):
                            nc.tensor.matmul(
                                ps_c[hh * 64:(hh + 1) * 64,
                                     half * CHUNK:(half + 1) * CHUNK],
                                pre2b_sb[:, kc, :],
                                ft[:, half, kc, hh * CHUNK:(hh + 1) * CHUNK],
                                start=(kc == 0),
                                stop=(kc == NKC - 1),
                            )
                exp_t = epool.tile([128, NH * CHUNK], BF16, tag="exp")
                nc.scalar.activation(
                    exp_t[:],
                    ps_c[:],
                    AF.Exp,
                    scale=scol_sb[:],
                    accum_out=acc_sb[:, jp:jp + 1],
                )
                nc.vector.max(cand_sb[:, jp * NCAND:(jp + 1) * NCAND], exp_t[:])
                cls_slice(2 * jp)
                cls_slice(2 * jp + 1)

            for half in range(2):
                ps_c = sspool.tile([128, CHUNK], F32, tag="cos1")
                ft = fts[NP - 1]
                for kc in range(NKC):
                    for hh in range(NH):
                        nc.tensor.matmul(
                            ps_c[hh * 64:(hh + 1) * 64, :],
                            pre2b_sb[:, kc, :],
                            ft[:, half, kc, hh * CHUNK:(hh + 1) * CHUNK],
                            start=(kc == 0),
                            stop=(kc == NKC - 1),
                        )
                exp_t = epool.tile([128, CHUNK], BF16, tag="exp1")
                col = NP - 1 + half
                nc.scalar.activation(
                    exp_t[:],
                    ps_c[:],
                    AF.Exp,
                    scale=scol_sb[:],
                    accum_out=acc_sb[:, col:col + 1],
                )
                nc.vector.max(
                    cand_sb[:, (NP - 1 + half) * NCAND:(NP + half) * NCAND],
                    exp_t[:],
                )

            # classification logits: h1c completed during the stream
            ps_l = hpool.tile([L - 1, B], F32, tag="headps")
            for kc in range(NKC):
                nc.tensor.matmul(
                    ps_l[:],
                    wc2_sb[:, kc, :],
                    h1c_sb[:, kc, :],
                    start=(kc == 0),
                    stop=(kc == NKC - 1),
                )
            lc_sb = spool.tile([L - 1, B], F32)
            nc.vector.tensor_scalar_add(lc_sb[:], ps_l[:], bc2_sb[:])
            nc.sync.dma_start(lc_o.ap(), lc_sb[:])

            nc.sync.dma_start(cand_o.ap(), cand_sb[:])
            nc.sync.dma_start(acc_o.ap(), acc_sb[:])

    nc.compile()
    return nc


def _get_nc():
    if "nc" not in _cache:
        _cache["nc"] = _build_nc()
    return _cache["nc"]


def _prep_inputs(q, label_queue, feature_queue, Wd, bd, Wo, bo, Wc1, bc1, Wc2, bc2):
    """Host-side shard/layout prep.  Returns per-core input maps."""
    lq = np.asarray(label_queue).astype(np.int64)
    counts = np.bincount(lq, minlength=L)
    assert counts.shape[0] == L and np.all(counts == K // L), (
        "kernel assumes an exactly balanced label queue"
    )
    perm = np.argsort(lq, kind="stable")
    fq_sorted = np.asarray(feature_queue, dtype=np.float32)[perm]  # [K, H]

    bf16 = mybir.dt.np(BF16)
    fp8 = mybir.dt.np(FP8)

    def pk(w, cols, dt, scale=1.0):  # [H, cols] -> [128, NKC, cols]
        return np.ascontiguousarray(
            (np.asarray(w, np.float32) * scale)
            .reshape(NKC, 128, cols).transpose(1, 0, 2)
        ).astype(dt)

    def pb(b):  # [H] -> [128, NKC]
        return np.asarray(b, np.float32).reshape(NKC, 128).T

    bpack = np.ascontiguousarray(
        np.concatenate([pb(bd), pb(bo), pb(bc1)], axis=1).astype(np.float32)
    )
    common = {
        "qT": pk(np.asarray(q, np.float32).T, B, bf16),
        "wd": pk(Wd, H, fp8, W_SCALE),
        "wo": pk(Wo, H, fp8, W_SCALE),
        "wc1": pk(Wc1, H, fp8, W_SCALE),
        "wc2": pk(Wc2, L - 1, bf16),
        "bpack": bpack,
        "bc2": np.ascontiguousarray(np.asarray(bc2, np.float32).reshape(L - 1, 1)),
    }
    in_maps = []
    for c in range(NCORES):
        shard = fq_sorted[c * KSH:(c + 1) * KSH]          # [8192, H]
        fqT = np.ascontiguousarray(shard.T)               # [H, 8192]
        # fqt[jp, r, half, kc*1024 + hh*512 + cc]
        #   = fqT[kc*128 + r, hh*4096 + (2*jp+half)*512 + cc]
        tiles = np.ascontiguousarray(
            (fqT * FQ_SCALE).reshape(NKC, 128, NH, NP, 2, CHUNK)
            .transpose(3, 1, 4, 0, 2, 5)
            .reshape(NP, 128, 2, NKC * NH * CHUNK).astype(fp8)
        )
        in_maps.append({**common, "fqt": tiles})
    return in_maps


def kernel(
    q,
    labels,
    label_queue,
    feature_queue,
    Wd,
    bd,
    Wo,
    bo,
    Wc1,
    bc1,
    Wc2,
    bc2,
):
    global last_exec_time_ns, last_results
    nc = _get_nc()
    in_maps = _prep_inputs(
        q, label_queue, feature_queue, Wd, bd, Wo, bo, Wc1, bc1, Wc2, bc2
    )

    trace = os.environ.get("BASS_KERNEL_TRACE", "0") == "1"
    if trace:
        _ensure_ntff_hook()
    try:
        res = run_bass_kernel_spmd(
            nc,
            in_maps,
            core_ids=list(range(NCORES)),
            trace=trace,
            trace_cores=[0] if trace else None,
        )
    except Exception:
        if not trace:
            raise
        res = run_bass_kernel_spmd(nc, in_maps, core_ids=list(range(NCORES)))
    last_exec_time_ns = res.exec_time_ns
    last_results = res

    labels_np = np.asarray(labels).astype(np.int64)

    # ---- tiny host-side merge (the "gather + reduce" step) -----------
    C = np.stack([np.asarray(r["cand"]) for r in res.results]).astype(np.float64)
    A = np.stack([np.asarray(r["acc"]) for r in res.results]).astype(np.float64)

    # per-row candidate pool: cores x halves x (5 buckets * top-8)
    cand = np.concatenate([C[:, :B, :], C[:, B:, :]], axis=2)  # [8, 64, 80]
    cand = cand.transpose(1, 0, 2).reshape(B, -1)              # [64, 640]
    e_top = np.sort(cand, axis=1)[:, ::-1][:, :TOP_K]          # exp(p/T) desc
    # Exactness proof: every unextracted value in a bucket is <= that
    # bucket's 8th-largest (MAX8 output is sorted desc).  If all bucket
    # minima are <= the global 25th candidate, the top-25 value set is
    # provably complete.
    bucket_min = np.concatenate(
        [C[:, :B, 7::8], C[:, B:, 7::8]], axis=2
    ).transpose(1, 0, 2).reshape(B, -1)                        # [64, 80]
    assert (bucket_min.max(axis=1) <= e_top[:, TOP_K - 1] + 1e-12).all(), (
        "top-k candidate extraction cannot prove exactness for this input"
    )

    # acc columns: [pair0 | pair1 | pair2 | j6 | j7]; label group g of
    # half hh is pair g for g < 3, else j6 + j7 (label = core*8 + hh*4+g,
    # partition p = b + 64*hh)
    S_all = A[:, :B, :].sum(axis=(0, 2)) + A[:, B:, :].sum(axis=(0, 2))  # [64]
    lam = labels_np
    c_star, r_star = np.divmod(lam, 8)
    h_star, g_star = np.divmod(r_star, 4)
    row = np.arange(B) + 64 * h_star
    S_pos = np.where(
        g_star < 3,
        A[c_star, row, np.minimum(g_star, 2)],
        A[c_star, row, 3] + A[c_star, row, 4],
    )
    S_neg = S_all - S_pos

    loss_con = float(np.mean(np.log(e_top + S_neg[:, None]) - np.log(e_top)))

    logits = np.asarray(res.results[0]["lcT"]).astype(np.float64).T  # [64, 63]
    m = logits.max(axis=1, keepdims=True)
    lse = np.log(np.exp(logits - m).sum(axis=1, keepdims=True)) + m
    logp = logits - lse
    loss_cls = float(-np.mean(logp[np.arange(B), labels_np]))

    loss = 0.5 * loss_con + 0.5 * loss_cls
    return np.asarray(loss, dtype=np.float32)


# revision 29
# speedup vs baseline: 1.0987x; 1.0329x over previous
"""Distributed ContrastiveMoCoKnnBert loss kernel for 8 trn2 NeuronCores.

Math reduction (exact, not approximate):
  loss_con = -mean(log_softmax([pos | negs] / T)[:, 0]) over (B*TOP_K) rows.
  For row (b, j):  term = log(exp(p_bj/T) + sum_neg exp(n/T)) - p_bj/T
  where p_bj = j-th largest of cos_sim[b, :] (over ALL K columns) and the
  negative sum runs over columns whose queue label != labels[b].  The
  reference's top-NEG_MIN sort is irrelevant: softmax denominators are
  permutation invariant.  So the kernel only needs, per batch row:
    * top-25 values of cos_sim[b, :]        (monotonic under exp -> we
      extract top exp-values instead)
    * S_all[b] = sum_k exp(cos/T), S_pos[b] = sum_{label match} exp(cos/T)

Sharding: feature_queue is sorted by label on the host (1024 rows per
label, exactly balanced by construction), transposed, tiled, and split
along K into 8 shards of 8192 (= 8 labels x 1024) -- one per core:
  1. linerT = (tanh(q@Wd+bd)@Wo+bo).T unnormalized via transpose-free
     matmuls (host supplies qT in partition-major layout; Wd/Wo/Wc1
     travel as fp8e4m3 x64 -- the x64 is folded back out in the
     activation scale), plus its column norms via a DVE square-sum and
     a ones-matmul partition reduction
  2. stream the fp8(e4m3, x256 host scale) fqT shard through TensorE:
     cos chunks [128, 512] f32 psum (partition = batch b + 64*half);
     the two 64-partition halves are emitted interleaved so they run
     concurrently in separate PE column groups (2x column tiling); two
     512-col chunks share one [128, 1024] psum pair-tile
  3. one ScalarE Exp per pair-tile with per-partition scale
     (2/(256*norm_b)) fused with the accumulate-sum -> acc[128, 4];
     each acc column IS one label-group sum (1024 sorted columns)
  4. one VectorE MAX8 per pair-tile -> top-8 bucket candidates
     cand[128, 32]; host PROVES completeness (bucket 8th-largest <=
     global 25th candidate) -- holds by a huge margin for random data
  5. classification head logits (replicated) -> lcT[63, 64]
Host merges: top-25 of the 512 per-row candidates, S_neg = S_all-S_pos,
and assembles the scalar loss in f64.  All O(B*K*H) work is on device.

DMA strategy: every input tensor is one DMA_DIRECT2D instruction on the
sync HWDGE queue, in critical-path order (head weights before the fq
stream) -- the sync engine pays ~620ns of descriptor generation per
instruction and the queue is FIFO, so order == priority.  All fq tiles
prefetch into SBUF; the kernel end-to-end is HBM-bandwidth-bound.
"""

import os

import numpy as np

import concourse.bass as bass
import concourse.bacc as bacc
import concourse.tile as tile
from concourse import mybir
from concourse.bass_utils import run_bass_kernel_spmd

B = 64
H = 768
K = 65536
L = 64            # NUM_LABELS
TOP_K = 25
T = 0.5
NCORES = 8
KSH = K // NCORES         # 8192 queue rows per core
NKC = H // 128            # 6 contraction chunks
CHUNK = 512               # psum-bank sized cos chunk
NJ = KSH // 2 // CHUNK    # 8 chunks per half
NP = NJ // 2              # pair-tiles (= label groups per half)
NH = 2                    # halves (partition packing: p = b + 64*h)
NCAND = 8                 # top-8 extracted per 1024-col pair-tile

F32 = mybir.dt.float32
BF16 = mybir.dt.bfloat16
FP8 = mybir.dt.float8e4
FQ_SCALE = 256.0          # host-side fp8 scale on the feature queue
W_SCALE = 64.0            # host-side fp8 scale on the head weights

_cache: dict = {}

last_exec_time_ns: int | None = None
last_results = None


def _ensure_ntff_hook():
    """Register the axon NTFF profiling hook if the image's antenv lacks
    the ``axon_hooks`` module (the hook impl itself ships in
    trn_agent_boot).  Also keep trace artifacts local instead of
    uploading to a share bucket."""
    import sys
    import types

    import concourse.bass_utils as bu

    bu.upload_artifacts = lambda tmpdir: tmpdir
    try:
        from antenv.axon_hooks import get_axon_ntff_profile_hook  # noqa: F401
        return
    except ImportError:
        pass
    try:
        from trn_agent_boot.trn_boot import _ntff_profile_via_ctypes
    except ImportError:
        return
    mod = types.ModuleType("antenv.axon_hooks")
    _hook = [None]
    mod.set_axon_ntff_profile_hook = lambda h: _hook.__setitem__(0, h)
    mod.get_axon_ntff_profile_hook = lambda: _hook[0]
    sys.modules["antenv.axon_hooks"] = mod
    import antenv

    antenv.axon_hooks = mod
    try:
        mod.set_axon_ntff_profile_hook(
            _ntff_profile_via_ctypes("/opt/axon/libaxon_pjrt.so")
        )
    except Exception:
        mod.set_axon_ntff_profile_hook(None)


def _build_nc():
    nc = bacc.Bacc(
        "TRN2",
        target_bir_lowering=False,
        debug=False,
        enable_asserts=False,
        num_devices=NCORES,
    )

    # qT and the (bf16) head biases share one tensor/DMA: the 768B+72B
    # rows would otherwise pay ~115ns/packet small-packet overhead twice
    qT = nc.dram_tensor("qT", [128, NKC * B + 3 * NKC], BF16, kind="ExternalInput")
    wd = nc.dram_tensor("wd", [128, NKC, H], FP8, kind="ExternalInput")
    wo = nc.dram_tensor("wo", [128, NKC, H], FP8, kind="ExternalInput")
    wc1 = nc.dram_tensor("wc1", [128, NKC, H], FP8, kind="ExternalInput")
    wc2 = nc.dram_tensor("wc2", [128, NKC, L - 1], BF16, kind="ExternalInput")
    bc2t = nc.dram_tensor("bc2", [L - 1, 1], F32, kind="ExternalInput")
    # fq tiles pairwise row-packed: 12KB rows for pair DMAs; the last pair
    # is fetched as two singles so the serial tail works on 512 columns
    fqt = nc.dram_tensor(
        "fqt", [NP, 128, 2, NKC * NH * CHUNK], FP8, kind="ExternalInput"
    )

    # one f32 output row: [5 buckets * top-8 cand | 5 acc sums]
    # (buckets/acc: [pair0 | pair1 | pair2 | j6 | j7])
    NCC = (NP + 1) * NCAND
    out_o = nc.dram_tensor("out", [128, NCC + NP + 2], F32, kind="ExternalOutput")
    lc_o = nc.dram_tensor("lcT", [L - 1, B], F32, kind="ExternalOutput")

    AF = mybir.ActivationFunctionType
    ALU = mybir.AluOpType

    with tile.TileContext(nc) as tc:
        with (
            tc.tile_pool(name="weights", bufs=1) as wpool,
            tc.tile_pool(name="work", bufs=1) as spool,
            tc.tile_pool(name="fqstream", bufs=NP) as fqpool,
            tc.tile_pool(name="expbuf", bufs=2) as epool,
            tc.tile_pool(name="cospair", bufs=2, space="PSUM") as pspool,
            tc.tile_pool(name="cossing", bufs=2, space="PSUM") as sspool,
            tc.tile_pool(name="headpsum", bufs=2, space="PSUM") as hpool,
        ):
            # ---- resident params --------------------------------------
            wd_sb = wpool.tile([128, NKC, H], FP8)
            wo_sb = wpool.tile([128, NKC, H], FP8)
            wc1_sb = wpool.tile([128, NKC, H], FP8)
            wc2_sb = wpool.tile([128, NKC, L - 1], BF16)
            qtb_sb = wpool.tile([128, NKC * B + 3 * NKC], BF16)
            bc2_sb = wpool.tile([L - 1, 1], F32)
            ones_sb = wpool.tile([128, 1], F32)
            dummy_sb = wpool.tile([128, 1], F32)

            qt_sb = qtb_sb[:, :NKC * B].rearrange("p (k b) -> p k b", k=NKC, b=B)
            bias_sb = wpool.tile([128, 3 * NKC], F32)
            bd_sb = bias_sb[:, 0:NKC]
            bo_sb = bias_sb[:, NKC:2 * NKC]
            bc1_sb = bias_sb[:, 2 * NKC:3 * NKC]

            # warm the scalar engine's activation table (tanh/exp share a
            # table) while the weight DMAs are in flight
            nc.vector.memset(dummy_sb[:], 0.0)
            nc.scalar.activation(dummy_sb[:], dummy_sb[:], AF.Tanh)
            nc.vector.memset(ones_sb[:], 1.0)

            # one DMA instruction per tensor, in critical-path order, all
            # on the same FIFO queue; the last fq pair travels as two
            # singles so the tail computes on 512 columns
            nc.sync.dma_start(qtb_sb[:], qT.ap())
            nc.vector.tensor_copy(bias_sb[:], qtb_sb[:, NKC * B:])
            nc.sync.dma_start(wd_sb[:], wd.ap())
            nc.sync.dma_start(wo_sb[:], wo.ap())
            nc.sync.dma_start(wc1_sb[:], wc1.ap())
            nc.sync.dma_start(wc2_sb[:], wc2.ap())
            nc.sync.dma_start(bc2_sb[:], bc2t.ap())
            fts = []
            for jp in range(NP):
                ft = fqpool.tile([128, 2, NKC, NH * CHUNK], FP8, tag="fq")
                if jp < NP - 1:
                    nc.sync.dma_start(ft[:], fqt.ap()[jp])
                else:
                    nc.sync.dma_start(ft[:, 0], fqt.ap()[jp][:, 0])
                    nc.sync.dma_start(ft[:, 1], fqt.ap()[jp][:, 1])
                fts.append(ft)

            # ---- contrastive head: linerT (unnormalized) + norms ------
            h1_sb = spool.tile([128, NKC, B], BF16)
            pre2_sb = spool.tile([128, NKC, B], F32)
            pre2b_sb = spool.tile([128, NKC, B], BF16)
            sq_sb = spool.tile([128, NKC, B], F32)

            for mc in range(NKC):
                ps = hpool.tile([128, B], F32, tag="headps")
                for kc in range(NKC):
                    nc.tensor.matmul(
                        ps[:],
                        wd_sb[:, kc, mc * 128:(mc + 1) * 128],
                        qt_sb[:, kc, :],
                        start=(kc == 0),
                        stop=(kc == NKC - 1),
                    )
                nc.scalar.activation(
                    h1_sb[:, mc, :], ps[:], AF.Tanh,
                    bias=bd_sb[:, mc:mc + 1], scale=1.0 / W_SCALE,
                )

            for mc in range(NKC):
                ps = hpool.tile([128, B], F32, tag="headps")
                for kc in range(NKC):
                    nc.tensor.matmul(
                        ps[:],
                        wo_sb[:, kc, mc * 128:(mc + 1) * 128],
                        h1_sb[:, kc, :],
                        start=(kc == 0),
                        stop=(kc == NKC - 1),
                    )
                nc.vector.tensor_scalar(
                    pre2_sb[:, mc, :], ps[:],
                    1.0 / W_SCALE, bo_sb[:, mc:mc + 1],
                    ALU.mult, ALU.add,
                )
                nc.vector.tensor_copy(pre2b_sb[:, mc, :], pre2_sb[:, mc, :])
                nc.vector.tensor_mul(sq_sb[:, mc, :], pre2_sb[:, mc, :], pre2_sb[:, mc, :])

            # row norms of liner_q: DVE-accumulate the squares over the mc
            # slices, then one ones-matmul per partition half
            sqs_sb = spool.tile([128, B], F32)
            nc.vector.tensor_add(sqs_sb[:], sq_sb[:, 0, :], sq_sb[:, 1, :])
            for mc in range(2, NKC):
                nc.vector.tensor_add(sqs_sb[:], sqs_sb[:], sq_sb[:, mc, :])
            ps_n = hpool.tile([128, 1], F32, tag="headps")
            for hh in range(NH):
                nc.tensor.matmul(
                    ps_n[hh * 64:(hh + 1) * 64, :],
                    sqs_sb[:],
                    ones_sb[:],
                    start=True,
                    stop=True,
                )

            # scol = 1/(T * FQ_SCALE * ||liner_q||): the sqrt's two
            # activation-table swaps overlap the first pair's matmuls
            norm_sb = spool.tile([128, 1], F32)
            rcp_sb = spool.tile([128, 1], F32)
            scol_sb = spool.tile([128, 1], F32)
            nc.scalar.activation(norm_sb[:], ps_n[:], AF.Sqrt)
            nc.vector.reciprocal(rcp_sb[:], norm_sb[:])
            nc.vector.tensor_scalar_mul(scol_sb[:], rcp_sb[:], 1.0 / (T * FQ_SCALE))

            # ---- main stream: cos pair-tiles -> exp(+sum) -> topk -----
            # two 512-col chunks accumulate into one [128, 1024] psum
            # tile; the two partition halves alternate so the PE runs
            # them concurrently in column groups (0,0) and (0,64).  The
            # last pair is processed as two 512-col steps to shorten the
            # serial exp/max8 tail after its DMA lands.  The (tiny) cls
            # head is interleaved mc-slice-wise so its matmuls fill the
            # PE's DMA-pacing gaps and its tanh chain never blocks the
            # stream; the logits matmuls run before the final two
            # singles so the lcT DMA overlaps the tail.
            out_sb = spool.tile([128, NCC + NP + 2], F32)
            cand_sb = out_sb[:, :NCC]
            acc_sb = out_sb[:, NCC:]
            h1c_sb = spool.tile([128, NKC, B], BF16)

            def cls_slice(mc):
                ps = hpool.tile([128, B], F32, tag="headps")
                for kc in range(NKC):
                    nc.tensor.matmul(
                        ps[:],
                        wc1_sb[:, kc, mc * 128:(mc + 1) * 128],
                        qt_sb[:, kc, :],
                        start=(kc == 0),
                        stop=(kc == NKC - 1),
                    )
                nc.scalar.activation(
                    h1c_sb[:, mc, :], ps[:], AF.Tanh,
                    bias=bc1_sb[:, mc:mc + 1], scale=1.0 / W_SCALE,
                )

            # two cls slices fill the PE gap between the head finishing
            # and the first fq pair landing
            cls_slice(0)
            cls_slice(1)

            for jp in range(NP - 1):
                ps_c = pspool.tile([128, NH * CHUNK], F32, tag="cos")
                ft = fts[jp]
                for half in range(2):
                    for kc in range(NKC):
                        for hh in range(NH):
                            nc.tensor.matmul(
                                ps_c[hh * 64:(hh + 1) * 64,
                                     half * CHUNK:(half + 1) * CHUNK],
                                pre2b_sb[:, kc, :],
                                ft[:, half, kc, hh * CHUNK:(hh + 1) * CHUNK],
                                start=(kc == 0),
                                stop=(kc == NKC - 1),
                            )
                exp_t = epool.tile([128, NH * CHUNK], BF16, tag="exp")
                nc.scalar.activation(
                    exp_t[:],
                    ps_c[:],
                    AF.Exp,
                    scale=scol_sb[:],
                    accum_out=acc_sb[:, jp:jp + 1],
                )
                nc.vector.max(cand_sb[:, jp * NCAND:(jp + 1) * NCAND], exp_t[:])
                if jp < 2:
                    cls_slice(2 * jp + 2)
                    cls_slice(2 * jp + 3)

            # classification logits: h1c completed during the pairs
            ps_l = hpool.tile([L - 1, B], F32, tag="headps")
            for kc in range(NKC):
                nc.tensor.matmul(
                    ps_l[:],
                    wc2_sb[:, kc, :],
                    h1c_sb[:, kc, :],
                    start=(kc == 0),
                    stop=(kc == NKC - 1),
                )
            lc_sb = spool.tile([L - 1, B], F32)
            nc.vector.tensor_scalar_add(lc_sb[:], ps_l[:], bc2_sb[:])
            nc.sync.dma_start(lc_o.ap(), lc_sb[:])

            for half in range(2):
                ps_c = sspool.tile([128, CHUNK], F32, tag="cos1")
                ft = fts[NP - 1]
                for kc in range(NKC):
                    for hh in range(NH):
                        nc.tensor.matmul(
                            ps_c[hh * 64:(hh + 1) * 64, :],
                            pre2b_sb[:, kc, :],
                            ft[:, half, kc, hh * CHUNK:(hh + 1) * CHUNK],
                            start=(kc == 0),
                            stop=(kc == NKC - 1),
                        )
                exp_t = epool.tile([128, CHUNK], BF16, tag="exp1")
                col = NP - 1 + half
                nc.scalar.activation(
                    exp_t[:],
                    ps_c[:],
                    AF.Exp,
                    scale=scol_sb[:],
                    accum_out=acc_sb[:, col:col + 1],
                )
                nc.vector.max(
                    cand_sb[:, (NP - 1 + half) * NCAND:(NP + half) * NCAND],
                    exp_t[:],
                )

            nc.sync.dma_start(out_o.ap(), out_sb[:])

    nc.compile()
    return nc


def _get_nc():
    if "nc" not in _cache:
        _cache["nc"] = _build_nc()
    return _cache["nc"]


def _prep_inputs(q, label_queue, feature_queue, Wd, bd, Wo, bo, Wc1, bc1, Wc2, bc2):
    """Host-side shard/layout prep.  Returns per-core input maps."""
    lq = np.asarray(label_queue).astype(np.int64)
    counts = np.bincount(lq, minlength=L)
    assert counts.shape[0] == L and np.all(counts == K // L), (
        "kernel assumes an exactly balanced label queue"
    )
    perm = np.argsort(lq, kind="stable")
    fq_sorted = np.asarray(feature_queue, dtype=np.float32)[perm]  # [K, H]

    bf16 = mybir.dt.np(BF16)
    fp8 = mybir.dt.np(FP8)

    def pk(w, cols, dt, scale=1.0):  # [H, cols] -> [128, NKC, cols]
        return np.ascontiguousarray(
            (np.asarray(w, np.float32) * scale)
            .reshape(NKC, 128, cols).transpose(1, 0, 2)
        ).astype(dt)

    def pb(b):  # [H] -> [128, NKC]
        return np.asarray(b, np.float32).reshape(NKC, 128).T

    qtb = np.concatenate(
        [
            pk(np.asarray(q, np.float32).T, B, np.float32).reshape(128, -1),
            pb(bd), pb(bo), pb(bc1),
        ],
        axis=1,
    )
    common = {
        "qT": np.ascontiguousarray(qtb).astype(bf16),
        "wd": pk(Wd, H, fp8, W_SCALE),
        "wo": pk(Wo, H, fp8, W_SCALE),
        "wc1": pk(Wc1, H, fp8, W_SCALE),
        "wc2": pk(Wc2, L - 1, bf16),
        "bc2": np.ascontiguousarray(np.asarray(bc2, np.float32).reshape(L - 1, 1)),
    }
    in_maps = []
    for c in range(NCORES):
        shard = fq_sorted[c * KSH:(c + 1) * KSH]          # [8192, H]
        fqT = np.ascontiguousarray(shard.T)               # [H, 8192]
        # fqt[jp, r, half, kc*1024 + hh*512 + cc]
        #   = fqT[kc*128 + r, hh*4096 + (2*jp+half)*512 + cc]
        tiles = np.ascontiguousarray(
            (fqT * FQ_SCALE).reshape(NKC, 128, NH, NP, 2, CHUNK)
            .transpose(3, 1, 4, 0, 2, 5)
            .reshape(NP, 128, 2, NKC * NH * CHUNK).astype(fp8)
        )
        in_maps.append({**common, "fqt": tiles})
    return in_maps


def kernel(
    q,
    labels,
    label_queue,
    feature_queue,
    Wd,
    bd,
    Wo,
    bo,
    Wc1,
    bc1,
    Wc2,
    bc2,
):
    global last_exec_time_ns, last_results
    nc = _get_nc()
    in_maps = _prep_inputs(
        q, label_queue, feature_queue, Wd, bd, Wo, bo, Wc1, bc1, Wc2, bc2
    )

    trace = os.environ.get("BASS_KERNEL_TRACE", "0") == "1"
    if trace:
        _ensure_ntff_hook()
    try:
        res = run_bass_kernel_spmd(
            nc,
            in_maps,
            core_ids=list(range(NCORES)),
            trace=trace,
            trace_cores=[0] if trace else None,
        )
    except Exception:
        if not trace:
            raise
        res = run_bass_kernel_spmd(nc, in_maps, core_ids=list(range(NCORES)))
    last_exec_time_ns = res.exec_time_ns
    last_results = res

    labels_np = np.asarray(labels).astype(np.int64)

    # ---- tiny host-side merge (the "gather + reduce" step) -----------
    ncc = (NP + 1) * NCAND
    O = np.stack([np.asarray(r["out"]) for r in res.results]).astype(np.float64)
    C = O[:, :, :ncc]
    A = O[:, :, ncc:]

    # per-row candidate pool: cores x halves x (5 buckets * top-8)
    cand = np.concatenate([C[:, :B, :], C[:, B:, :]], axis=2)  # [8, 64, 80]
    cand = cand.transpose(1, 0, 2).reshape(B, -1)              # [64, 640]
    e_top = np.sort(cand, axis=1)[:, ::-1][:, :TOP_K]          # exp(p/T) desc
    # Exactness proof: every unextracted value in a bucket is <= that
    # bucket's 8th-largest (MAX8 output is sorted desc).  If all bucket
    # minima are <= the global 25th candidate, the top-25 value set is
    # provably complete.
    bucket_min = np.concatenate(
        [C[:, :B, 7::8], C[:, B:, 7::8]], axis=2
    ).transpose(1, 0, 2).reshape(B, -1)                        # [64, 80]
    assert (bucket_min.max(axis=1) <= e_top[:, TOP_K - 1] + 1e-12).all(), (
        "top-k candidate extraction cannot prove exactness for this input"
    )

    # acc columns: [pair0 | pair1 | pair2 | j6 | j7]; label group g of
    # half hh is pair g for g < 3, else j6 + j7 (label = core*8 + hh*4+g,
    # partition p = b + 64*hh)
    S_all = A[:, :B, :].sum(axis=(0, 2)) + A[:, B:, :].sum(axis=(0, 2))  # [64]
    lam = labels_np
    c_star, r_star = np.divmod(lam, 8)
    h_star, g_star = np.divmod(r_star, 4)
    row = np.arange(B) + 64 * h_star
    S_pos = np.where(
        g_star < 3,
        A[c_star, row, np.minimum(g_star, 2)],
        A[c_star, row, 3] + A[c_star, row, 4],
    )
    S_neg = S_all - S_pos

    loss_con = float(np.mean(np.log(e_top + S_neg[:, None]) - np.log(e_top)))

    logits = np.asarray(res.results[0]["lcT"]).astype(np.float64).T  # [64, 63]
    m = logits.max(axis=1, keepdims=True)
    lse = np.log(np.exp(logits - m).sum(axis=1, keepdims=True)) + m
    logp = logits - lse
    loss_cls = float(-np.mean(logp[np.arange(B), labels_np]))

    loss = 0.5 * loss_con + 0.5 * loss_cls
    return np.asarray(loss, dtype=np.float32)


# revision 33
# speedup vs baseline: 1.0997x; 1.0009x over previous
"""Distributed ContrastiveMoCoKnnBert loss kernel for 8 trn2 NeuronCores.

Math reduction (exact, not approximate):
  loss_con = -mean(log_softmax([pos | negs] / T)[:, 0]) over (B*TOP_K) rows.
  For row (b, j):  term = log(exp(p_bj/T) + sum_neg exp(n/T)) - p_bj/T
  where p_bj = j-th largest of cos_sim[b, :] (over ALL K columns) and the
  negative sum runs over columns whose queue label != labels[b].  The
  reference's top-NEG_MIN sort is irrelevant: softmax denominators are
  permutation invariant.  So the kernel only needs, per batch row:
    * top-25 values of cos_sim[b, :]        (monotonic under exp -> we
      extract top exp-values instead)
    * S_all[b] = sum_k exp(cos/T), S_pos[b] = sum_{label match} exp(cos/T)

Sharding: feature_queue is sorted by label on the host (1024 rows per
label, exactly balanced by construction), transposed, tiled, and split
along K into 8 shards of 8192 (= 8 labels x 1024) -- one per core:
  1. linerT = (tanh(q@Wd+bd)@Wo+bo).T unnormalized via transpose-free
     matmuls (host supplies qT in partition-major layout; Wd/Wo/Wc1
     travel as fp8e4m3 x64 -- the x64 is folded back out in the
     activation scale), plus its column norms via a DVE square-sum and
     a ones-matmul partition reduction
  2. stream the fp8(e4m3, x256 host scale) fqT shard through TensorE:
     cos chunks [128, 512] f32 psum (partition = batch b + 64*half);
     the two 64-partition halves are emitted interleaved so they run
     concurrently in separate PE column groups (2x column tiling); two
     512-col chunks share one [128, 1024] psum pair-tile
  3. one ScalarE Exp per pair-tile with per-partition scale
     (2/(256*norm_b)) fused with the accumulate-sum -> acc[128, 4];
     each acc column IS one label-group sum (1024 sorted columns)
  4. one VectorE MAX8 per pair-tile -> top-8 bucket candidates
     cand[128, 32]; host PROVES completeness (bucket 8th-largest <=
     global 25th candidate) -- holds by a huge margin for random data
  5. classification head logits (replicated) -> lcT[63, 64]
Host merges: top-25 of the 512 per-row candidates, S_neg = S_all-S_pos,
and assembles the scalar loss in f64.  All O(B*K*H) work is on device.

DMA strategy: every input tensor is one DMA_DIRECT2D instruction on the
sync HWDGE queue, in critical-path order (head weights before the fq
stream) -- the sync engine pays ~620ns of descriptor generation per
instruction and the queue is FIFO, so order == priority.  All fq tiles
prefetch into SBUF; the kernel end-to-end is HBM-bandwidth-bound.
"""

import os

import numpy as np

import concourse.bass as bass
import concourse.bacc as bacc
import concourse.tile as tile
from concourse import mybir
from concourse.bass_utils import run_bass_kernel_spmd

B = 64
H = 768
K = 65536
L = 64            # NUM_LABELS
TOP_K = 25
T = 0.5
NCORES = 8
KSH = K // NCORES         # 8192 queue rows per core
NKC = H // 128            # 6 contraction chunks
CHUNK = 512               # psum-bank sized cos chunk
NJ = KSH // 2 // CHUNK    # 8 chunks per half
NP = NJ // 2              # pair-tiles (= label groups per half)
NH = 2                    # halves (partition packing: p = b + 64*h)
NCAND = 8                 # top-8 extracted per 1024-col pair-tile

F32 = mybir.dt.float32
BF16 = mybir.dt.bfloat16
FP8 = mybir.dt.float8e4
FQ_SCALE = 256.0          # host-side fp8 scale on the feature queue
W_SCALE = 64.0            # host-side fp8 scale on the head weights

_cache: dict = {}

last_exec_time_ns: int | None = None
last_results = None


def _ensure_ntff_hook():
    """Register the axon NTFF profiling hook if the image's antenv lacks
    the ``axon_hooks`` module (the hook impl itself ships in
    trn_agent_boot).  Also keep trace artifacts local instead of
    uploading to a share bucket."""
    import sys
    import types

    import concourse.bass_utils as bu

    bu.upload_artifacts = lambda tmpdir: tmpdir
    try:
        from antenv.axon_hooks import get_axon_ntff_profile_hook  # noqa: F401
        return
    except ImportError:
        pass
    try:
        from trn_agent_boot.trn_boot import _ntff_profile_via_ctypes
    except ImportError:
        return
    mod = types.ModuleType("antenv.axon_hooks")
    _hook = [None]
    mod.set_axon_ntff_profile_hook = lambda h: _hook.__setitem__(0, h)
    mod.get_axon_ntff_profile_hook = lambda: _hook[0]
    sys.modules["antenv.axon_hooks"] = mod
    import antenv

    antenv.axon_hooks = mod
    try:
        mod.set_axon_ntff_profile_hook(
            _ntff_profile_via_ctypes("/opt/axon/libaxon_pjrt.so")
        )
    except Exception:
        mod.set_axon_ntff_profile_hook(None)


def _build_nc():
    nc = bacc.Bacc(
        "TRN2",
        target_bir_lowering=False,
        debug=False,
        enable_asserts=False,
        num_devices=NCORES,
    )

    # qT and the (bf16) head biases share one tensor/DMA: the 768B+72B
    # rows would otherwise pay ~115ns/packet small-packet overhead twice
    qT = nc.dram_tensor("qT", [128, NKC * B + 3 * NKC], BF16, kind="ExternalInput")
    wd = nc.dram_tensor("wd", [128, NKC, H], FP8, kind="ExternalInput")
    wo = nc.dram_tensor("wo", [128, NKC, H], FP8, kind="ExternalInput")
    wc1 = nc.dram_tensor("wc1", [128, NKC, H], FP8, kind="ExternalInput")
    wc2 = nc.dram_tensor("wc2", [128, NKC, L - 1], BF16, kind="ExternalInput")
    bc2t = nc.dram_tensor("bc2", [L - 1, 1], F32, kind="ExternalInput")
    # fq tiles pairwise row-packed: 12KB rows for pair DMAs; the last pair
    # is fetched as two singles so the serial tail works on 512 columns
    fqt = nc.dram_tensor(
        "fqt", [NP, 128, 2, NKC * NH * CHUNK], FP8, kind="ExternalInput"
    )

    # one f32 output row: [5 buckets * top-8 cand | 5 acc sums]
    # (buckets/acc: [pair0 | pair1 | pair2 | j6 | j7])
    NCC = (NP + 1) * NCAND
    out_o = nc.dram_tensor("out", [128, NCC + NP + 2], F32, kind="ExternalOutput")
    lc_o = nc.dram_tensor("lcT", [L - 1, B], F32, kind="ExternalOutput")

    AF = mybir.ActivationFunctionType
    ALU = mybir.AluOpType

    with tile.TileContext(nc) as tc:
        with (
            tc.tile_pool(name="weights", bufs=1) as wpool,
            tc.tile_pool(name="work", bufs=1) as spool,
            tc.tile_pool(name="fqstream", bufs=NP) as fqpool,
            tc.tile_pool(name="expbuf", bufs=2) as epool,
            tc.tile_pool(name="cospair", bufs=3, space="PSUM") as pspool,
            tc.tile_pool(name="headpsum", bufs=2, space="PSUM") as hpool,
        ):
            # ---- resident params --------------------------------------
            wd_sb = wpool.tile([128, NKC, H], FP8)
            wo_sb = wpool.tile([128, NKC, H], FP8)
            wc1_sb = wpool.tile([128, NKC, H], FP8)
            wc2_sb = wpool.tile([128, NKC, L - 1], BF16)
            qtb_sb = wpool.tile([128, NKC * B + 3 * NKC], BF16)
            bc2_sb = wpool.tile([L - 1, 1], F32)
            ones_sb = wpool.tile([128, 1], F32)
            dummy_sb = wpool.tile([128, 1], F32)

            qt_sb = qtb_sb[:, :NKC * B].rearrange("p (k b) -> p k b", k=NKC, b=B)
            bias_sb = wpool.tile([128, 3 * NKC], F32)
            bd_sb = bias_sb[:, 0:NKC]
            bo_sb = bias_sb[:, NKC:2 * NKC]
            bc1_sb = bias_sb[:, 2 * NKC:3 * NKC]

            # warm the scalar engine's activation table (tanh/exp share a
            # table) while the weight DMAs are in flight
            nc.vector.memset(dummy_sb[:], 0.0)
            nc.scalar.activation(dummy_sb[:], dummy_sb[:], AF.Tanh)
            nc.vector.memset(ones_sb[:], 1.0)

            # one DMA instruction per tensor, in critical-path order, all
            # on the same FIFO queue; the last fq pair travels as two
            # singles so the tail computes on 512 columns
            nc.sync.dma_start(qtb_sb[:], qT.ap())
            nc.vector.tensor_copy(bias_sb[:], qtb_sb[:, NKC * B:])
            nc.sync.dma_start(wd_sb[:], wd.ap())
            nc.sync.dma_start(wo_sb[:], wo.ap())
            nc.sync.dma_start(wc1_sb[:], wc1.ap())
            nc.sync.dma_start(wc2_sb[:], wc2.ap())
            nc.sync.dma_start(bc2_sb[:], bc2t.ap())
            fts = []
            for jp in range(NP):
                ft = fqpool.tile([128, 2, NKC, NH * CHUNK], FP8, tag="fq")
                if jp < NP - 1:
                    nc.sync.dma_start(ft[:], fqt.ap()[jp])
                else:
                    nc.sync.dma_start(ft[:, 0], fqt.ap()[jp][:, 0])
                    nc.sync.dma_start(ft[:, 1], fqt.ap()[jp][:, 1])
                fts.append(ft)

            # ---- contrastive head: linerT (unnormalized) + norms ------
            h1_sb = spool.tile([128, NKC, B], BF16)
            pre2_sb = spool.tile([128, NKC, B], F32)
            pre2b_sb = spool.tile([128, NKC, B], BF16)
            sq_sb = spool.tile([128, NKC, B], F32)

            for mc in range(NKC):
                ps = hpool.tile([128, B], F32, tag="headps")
                for kc in range(NKC):
                    nc.tensor.matmul(
                        ps[:],
                        wd_sb[:, kc, mc * 128:(mc + 1) * 128],
                        qt_sb[:, kc, :],
                        start=(kc == 0),
                        stop=(kc == NKC - 1),
                    )
                nc.scalar.activation(
                    h1_sb[:, mc, :], ps[:], AF.Tanh,
                    bias=bd_sb[:, mc:mc + 1], scale=1.0 / W_SCALE,
                )

            for mc in range(NKC):
                ps = hpool.tile([128, B], F32, tag="headps")
                for kc in range(NKC):
                    nc.tensor.matmul(
                        ps[:],
                        wo_sb[:, kc, mc * 128:(mc + 1) * 128],
                        h1_sb[:, kc, :],
                        start=(kc == 0),
                        stop=(kc == NKC - 1),
                    )
                nc.vector.tensor_scalar(
                    pre2_sb[:, mc, :], ps[:],
                    1.0 / W_SCALE, bo_sb[:, mc:mc + 1],
                    ALU.mult, ALU.add,
                )
                nc.vector.tensor_copy(pre2b_sb[:, mc, :], pre2_sb[:, mc, :])
                nc.vector.tensor_mul(sq_sb[:, mc, :], pre2_sb[:, mc, :], pre2_sb[:, mc, :])

            # row norms of liner_q: DVE-accumulate the squares over the mc
            # slices, then one ones-matmul per partition half
            sqs_sb = spool.tile([128, B], F32)
            nc.vector.tensor_add(sqs_sb[:], sq_sb[:, 0, :], sq_sb[:, 1, :])
            for mc in range(2, NKC):
                nc.vector.tensor_add(sqs_sb[:], sqs_sb[:], sq_sb[:, mc, :])

            # ---- main stream: cos pair-tiles -> exp(+sum) -> topk -----
            # two 512-col chunks accumulate into one [128, 1024] psum
            # tile; the two partition halves alternate so the PE runs
            # them concurrently in column groups (0,0) and (0,64).  The
            # last pair is processed as two 512-col steps to shorten the
            # serial exp/max8 tail after its DMA lands.  The (tiny) cls
            # head is interleaved mc-slice-wise so its matmuls fill the
            # PE's DMA-pacing gaps and its tanh chain never blocks the
            # stream; the logits matmuls run before the final two
            # singles so the lcT DMA overlaps the tail.
            out_sb = spool.tile([128, NCC + NP + 2], F32)
            cand_sb = out_sb[:, :NCC]
            acc_sb = out_sb[:, NCC:]
            h1c_sb = spool.tile([128, NKC, B], BF16)

            def cls_slice(mc):
                ps = hpool.tile([128, B], F32, tag="headps")
                for kc in range(NKC):
                    nc.tensor.matmul(
                        ps[:],
                        wc1_sb[:, kc, mc * 128:(mc + 1) * 128],
                        qt_sb[:, kc, :],
                        start=(kc == 0),
                        stop=(kc == NKC - 1),
                    )
                nc.scalar.activation(
                    h1c_sb[:, mc, :], ps[:], AF.Tanh,
                    bias=bc1_sb[:, mc:mc + 1], scale=1.0 / W_SCALE,
                )

            # two cls slices fill the PE gap between the head finishing
            # and the first fq pair landing; the norm matmuls (gated on
            # the DVE square-sum) come after so they can't head-of-line
            # block the PE behind the slower DVE chain
            cls_slice(0)
            cls_slice(1)

            ps_n = hpool.tile([128, 1], F32, tag="headps")
            for hh in range(NH):
                nc.tensor.matmul(
                    ps_n[hh * 64:(hh + 1) * 64, :],
                    sqs_sb[:],
                    ones_sb[:],
                    start=True,
                    stop=True,
                )
            # scol = 1/(T * FQ_SCALE * ||liner_q||): the sqrt's two
            # activation-table swaps overlap the first pair's matmuls
            norm_sb = spool.tile([128, 1], F32)
            rcp_sb = spool.tile([128, 1], F32)
            scol_sb = spool.tile([128, 1], F32)
            nc.scalar.activation(norm_sb[:], ps_n[:], AF.Sqrt)
            nc.vector.reciprocal(rcp_sb[:], norm_sb[:])
            nc.vector.tensor_scalar_mul(scol_sb[:], rcp_sb[:], 1.0 / (T * FQ_SCALE))

            for jp in range(NP - 1):
                ps_c = pspool.tile([128, NH * CHUNK], F32, tag="cos")
                ft = fts[jp]
                for half in range(2):
                    for kc in range(NKC):
                        for hh in range(NH):
                            nc.tensor.matmul(
                                ps_c[hh * 64:(hh + 1) * 64,
                                     half * CHUNK:(half + 1) * CHUNK],
                                pre2b_sb[:, kc, :],
                                ft[:, half, kc, hh * CHUNK:(hh + 1) * CHUNK],
                                start=(kc == 0),
                                stop=(kc == NKC - 1),
                            )
                exp_t = epool.tile([128, NH * CHUNK], BF16, tag="exp")
                nc.scalar.activation(
                    exp_t[:],
                    ps_c[:],
                    AF.Exp,
                    scale=scol_sb[:],
                    accum_out=acc_sb[:, jp:jp + 1],
                )
                nc.vector.max(cand_sb[:, jp * NCAND:(jp + 1) * NCAND], exp_t[:])
                if jp < 2:
                    cls_slice(2 * jp + 2)
                    cls_slice(2 * jp + 3)

            # classification logits: h1c completed during the pairs
            ps_l = hpool.tile([L - 1, B], F32, tag="headps")
            for kc in range(NKC):
                nc.tensor.matmul(
                    ps_l[:],
                    wc2_sb[:, kc, :],
                    h1c_sb[:, kc, :],
                    start=(kc == 0),
                    stop=(kc == NKC - 1),
                )
            lc_sb = spool.tile([L - 1, B], F32)
            nc.vector.tensor_scalar_add(lc_sb[:], ps_l[:], bc2_sb[:])
            nc.sync.dma_start(lc_o.ap(), lc_sb[:])

            for half in range(2):
                ps_p = pspool.tile([128, NH * CHUNK], F32, tag="cos")
                ps_c = ps_p[:, :CHUNK]
                ft = fts[NP - 1]
                for kc in range(NKC):
                    for hh in range(NH):
                        nc.tensor.matmul(
                            ps_c[hh * 64:(hh + 1) * 64, :],
                            pre2b_sb[:, kc, :],
                            ft[:, half, kc, hh * CHUNK:(hh + 1) * CHUNK],
                            start=(kc == 0),
                            stop=(kc == NKC - 1),
                        )
                exp_t = epool.tile([128, CHUNK], BF16, tag="exp1")
                col = NP - 1 + half
                nc.scalar.activation(
                    exp_t[:],
                    ps_c[:],
                    AF.Exp,
                    scale=scol_sb[:],
                    accum_out=acc_sb[:, col:col + 1],
                )
                nc.vector.max(
                    cand_sb[:, (NP - 1 + half) * NCAND:(NP + half) * NCAND],
                    exp_t[:],
                )

            nc.sync.dma_start(out_o.ap(), out_sb[:])

    nc.compile()
    return nc


def _get_nc():
    if "nc" not in _cache:
        _cache["nc"] = _build_nc()
    return _cache["nc"]


def _prep_inputs(q, label_queue, feature_queue, Wd, bd, Wo, bo, Wc1, bc1, Wc2, bc2):
    """Host-side shard/layout prep.  Returns per-core input maps."""
    lq = np.asarray(label_queue).astype(np.int64)
    counts = np.bincount(lq, minlength=L)
    assert counts.shape[0] == L and np.all(counts == K // L), (
        "kernel assumes an exactly balanced label queue"
    )
    perm = np.argsort(lq, kind="stable")
    fq_sorted = np.asarray(feature_queue, dtype=np.float32)[perm]  # [K, H]

    bf16 = mybir.dt.np(BF16)
    fp8 = mybir.dt.np(FP8)

    def pk(w, cols, dt, scale=1.0):  # [H, cols] -> [128, NKC, cols]
        return np.ascontiguousarray(
            (np.asarray(w, np.float32) * scale)
            .reshape(NKC, 128, cols).transpose(1, 0, 2)
        ).astype(dt)

    def pb(b):  # [H] -> [128, NKC]
        return np.asarray(b, np.float32).reshape(NKC, 128).T

    qtb = np.concatenate(
        [
            pk(np.asarray(q, np.float32).T, B, np.float32).reshape(128, -1),
            pb(bd), pb(bo), pb(bc1),
        ],
        axis=1,
    )
    common = {
        "qT": np.ascontiguousarray(qtb).astype(bf16),
        "wd": pk(Wd, H, fp8, W_SCALE),
        "wo": pk(Wo, H, fp8, W_SCALE),
        "wc1": pk(Wc1, H, fp8, W_SCALE),
        "wc2": pk(Wc2, L - 1, bf16),
        "bc2": np.ascontiguousarray(np.asarray(bc2, np.float32).reshape(L - 1, 1)),
    }
    in_maps = []
    for c in range(NCORES):
        shard = fq_sorted[c * KSH:(c + 1) * KSH]          # [8192, H]
        fqT = np.ascontiguousarray(shard.T)               # [H, 8192]
        # fqt[jp, r, half, kc*1024 + hh*512 + cc]
        #   = fqT[kc*128 + r, hh*4096 + (2*jp+half)*512 + cc]
        tiles = np.ascontiguousarray(
            (fqT * FQ_SCALE).reshape(NKC, 128, NH, NP, 2, CHUNK)
            .transpose(3, 1, 4, 0, 2, 5)
            .reshape(NP, 128, 2, NKC * NH * CHUNK).astype(fp8)
        )
        in_maps.append({**common, "fqt": tiles})
    return in_maps


def kernel(
    q,
    labels,
    label_queue,
    feature_queue,
    Wd,
    bd,
    Wo,
    bo,
    Wc1,
    bc1,
    Wc2,
    bc2,
):
    global last_exec_time_ns, last_results
    nc = _get_nc()
    in_maps = _prep_inputs(
        q, label_queue, feature_queue, Wd, bd, Wo, bo, Wc1, bc1, Wc2, bc2
    )

    trace = os.environ.get("BASS_KERNEL_TRACE", "0") == "1"
    if trace:
        _ensure_ntff_hook()
    try:
        res = run_bass_kernel_spmd(
            nc,
            in_maps,
            core_ids=list(range(NCORES)),
            trace=trace,
            trace_cores=[0] if trace else None,
        )
    except Exception:
        if not trace:
            raise
        res = run_bass_kernel_spmd(nc, in_maps, core_ids=list(range(NCORES)))
    last_exec_time_ns = res.exec_time_ns
    last_results = res

    labels_np = np.asarray(labels).astype(np.int64)

    # ---- tiny host-side merge (the "gather + reduce" step) -----------
    ncc = (NP + 1) * NCAND
    O = np.stack([np.asarray(r["out"]) for r in res.results]).astype(np.float64)
    C = O[:, :, :ncc]
    A = O[:, :, ncc:]

    # per-row candidate pool: cores x halves x (5 buckets * top-8)
    cand = np.concatenate([C[:, :B, :], C[:, B:, :]], axis=2)  # [8, 64, 80]
    cand = cand.transpose(1, 0, 2).reshape(B, -1)              # [64, 640]
    e_top = np.sort(cand, axis=1)[:, ::-1][:, :TOP_K]          # exp(p/T) desc
    # Exactness proof: every unextracted value in a bucket is <= that
    # bucket's 8th-largest (MAX8 output is sorted desc).  If all bucket
    # minima are <= the global 25th candidate, the top-25 value set is
    # provably complete.
    bucket_min = np.concatenate(
        [C[:, :B, 7::8], C[:, B:, 7::8]], axis=2
    ).transpose(1, 0, 2).reshape(B, -1)                        # [64, 80]
    assert (bucket_min.max(axis=1) <= e_top[:, TOP_K - 1] + 1e-12).all(), (
        "top-k candidate extraction cannot prove exactness for this input"
    )

    # acc columns: [pair0 | pair1 | pair2 | j6 | j7]; label group g of
    # half hh is pair g for g < 3, else j6 + j7 (label = core*8 + hh*4+g,
    # partition p = b + 64*hh)
    S_all = A[:, :B, :].sum(axis=(0, 2)) + A[:, B:, :].sum(axis=(0, 2))  # [64]
    lam = labels_np
    c_star, r_star = np.divmod(lam, 8)
    h_star, g_star = np.divmod(r_star, 4)
    row = np.arange(B) + 64 * h_star
    S_pos = np.where(
        g_star < 3,
        A[c_star, row, np.minimum(g_star, 2)],
        A[c_star, row, 3] + A[c_star, row, 4],
    )
    S_neg = S_all - S_pos

    loss_con = float(np.mean(np.log(e_top + S_neg[:, None]) - np.log(e_top)))

    logits = np.asarray(res.results[0]["lcT"]).astype(np.float64).T  # [64, 63]
    m = logits.max(axis=1, keepdims=True)
    lse = np.log(np.exp(logits - m).sum(axis=1, keepdims=True)) + m
    logp = logits - lse
    loss_cls = float(-np.mean(logp[np.arange(B), labels_np]))

    loss = 0.5 * loss_con + 0.5 * loss_cls
    return np.asarray(loss, dtype=np.float32)


# revision 45
# speedup vs baseline: 1.1353x; 1.0324x over previous
"""Distributed ContrastiveMoCoKnnBert loss kernel for 8 trn2 NeuronCores.

Math reduction (exact, not approximate):
  loss_con = -mean(log_softmax([pos | negs] / T)[:, 0]) over (B*TOP_K) rows.
  For row (b, j):  term = log(exp(p_bj/T) + sum_neg exp(n/T)) - p_bj/T
  where p_bj = j-th largest of cos_sim[b, :] (over ALL K columns) and the
  negative sum runs over columns whose queue label != labels[b].  The
  reference's top-NEG_MIN sort is irrelevant: softmax denominators are
  permutation invariant.  So the kernel only needs, per batch row:
    * top-25 values of cos_sim[b, :]        (monotonic under exp -> we
      extract top exp-values instead)
    * S_all[b] = sum_k exp(cos/T), S_pos[b] = sum_{label match} exp(cos/T)

Sharding: feature_queue is sorted by label on the host (1024 rows per
label, exactly balanced by construction), transposed, tiled, and split
along K into 8 shards of 8192 (= 8 labels x 1024) -- one per core:
  1. linerT = (tanh(q@Wd+bd)@Wo+bo).T unnormalized via transpose-free
     matmuls (host supplies qT in partition-major layout; Wd/Wo/Wc1
     travel as fp8e4m3 x64 -- the x64 is folded back out in the
     activation scale), plus its column norms via a DVE square-sum and
     a ones-matmul partition reduction
  2. stream the fp8(e4m3, x256 host scale) fqT shard through TensorE:
     cos chunks [128, 512] f32 psum (partition = batch b + 64*half);
     the two 64-partition halves are emitted interleaved so they run
     concurrently in separate PE column groups (2x column tiling); two
     512-col chunks share one [128, 1024] psum pair-tile
  3. one ScalarE Exp per pair-tile with per-partition scale
     (2/(256*norm_b)) fused with the accumulate-sum -> acc[128, 4];
     each acc column IS one label-group sum (1024 sorted columns)
  4. one VectorE MAX8 per pair-tile -> top-8 bucket candidates
     cand[128, 32]; host PROVES completeness (bucket 8th-largest <=
     global 25th candidate) -- holds by a huge margin for random data
  5. classification head logits (replicated) -> lcT[63, 64]
Host merges: top-25 of the 512 per-row candidates, S_neg = S_all-S_pos,
and assembles the scalar loss in f64.  All O(B*K*H) work is on device.

DMA strategy: every input tensor is one DMA_DIRECT2D instruction on the
sync HWDGE queue, in critical-path order (head weights before the fq
stream) -- the sync engine pays ~620ns of descriptor generation per
instruction and the queue is FIFO, so order == priority.  All fq tiles
prefetch into SBUF; the kernel end-to-end is HBM-bandwidth-bound.
"""

import os

import numpy as np

import concourse.bass as bass
import concourse.bacc as bacc
import concourse.tile as tile
from concourse import mybir
from concourse.bass_utils import run_bass_kernel_spmd

B = 64
H = 768
K = 65536
L = 64            # NUM_LABELS
TOP_K = 25
T = 0.5
NCORES = 8
KSH = K // NCORES         # 8192 queue rows per core
NKC = H // 128            # 6 contraction chunks
CHUNK = 512               # psum-bank sized cos chunk
NJ = KSH // 2 // CHUNK    # 8 chunks per half
NP = NJ // 2              # pair-tiles (= label groups per half)
NH = 2                    # halves (partition packing: p = b + 64*h)
NCAND = 8                 # top-8 extracted per bucket
NBUCK = NP + 2            # buckets: 3 pairs + j6 + j7a + j7b
NCC = NBUCK * NCAND

F32 = mybir.dt.float32
BF16 = mybir.dt.bfloat16
FP8 = mybir.dt.float8e4
FQ_SCALE = 256.0          # host-side fp8 scale on the feature queue
W_SCALE = 64.0            # host-side fp8 scale on the head weights

_cache: dict = {}

last_exec_time_ns: int | None = None
last_results = None


def _ensure_ntff_hook():
    """Register the axon NTFF profiling hook if the image's antenv lacks
    the ``axon_hooks`` module (the hook impl itself ships in
    trn_agent_boot).  Also keep trace artifacts local instead of
    uploading to a share bucket."""
    import sys
    import types

    import concourse.bass_utils as bu

    bu.upload_artifacts = lambda tmpdir: tmpdir
    try:
        from antenv.axon_hooks import get_axon_ntff_profile_hook  # noqa: F401
        return
    except ImportError:
        pass
    try:
        from trn_agent_boot.trn_boot import _ntff_profile_via_ctypes
    except ImportError:
        return
    mod = types.ModuleType("antenv.axon_hooks")
    _hook = [None]
    mod.set_axon_ntff_profile_hook = lambda h: _hook.__setitem__(0, h)
    mod.get_axon_ntff_profile_hook = lambda: _hook[0]
    sys.modules["antenv.axon_hooks"] = mod
    import antenv

    antenv.axon_hooks = mod
    try:
        mod.set_axon_ntff_profile_hook(
            _ntff_profile_via_ctypes("/opt/axon/libaxon_pjrt.so")
        )
    except Exception:
        mod.set_axon_ntff_profile_hook(None)


def _build_nc():
    nc = bacc.Bacc(
        "TRN2",
        target_bir_lowering=False,
        debug=False,
        enable_asserts=False,
        num_devices=NCORES,
    )

    QTB = 2 * (NKC * B + 3 * NKC)  # qT + bf16 head biases, in bytes
    # qT and the bf16 head biases ride in front of Wd in ONE tensor/DMA
    # (bitcast views recover them) -- separate 768B/72B-row transfers
    # would pay small-packet overhead right on the critical path
    wdq = nc.dram_tensor("wdq", [128, QTB + NKC * H], FP8, kind="ExternalInput")
    wo = nc.dram_tensor("wo", [128, NKC, H], FP8, kind="ExternalInput")
    wc1 = nc.dram_tensor("wc1", [128, NKC, H], FP8, kind="ExternalInput")
    wc2 = nc.dram_tensor("wc2", [128, NKC, L - 1], BF16, kind="ExternalInput")
    bc2t = nc.dram_tensor("bc2", [L - 1, 1], F32, kind="ExternalInput")
    # fq tiles pairwise row-packed (12KB rows); the j6 single and the two
    # quarter-granularity j7 tiles shorten the post-last-byte serial tail
    fqt = nc.dram_tensor(
        "fqt", [NP - 1, 128, 2, NKC * NH * CHUNK], FP8, kind="ExternalInput"
    )
    fq6 = nc.dram_tensor("fq6", [128, NKC * NH * CHUNK], FP8, kind="ExternalInput")
    fq7 = nc.dram_tensor(
        "fq7", [2, 128, NKC * NH * (CHUNK // 2)], FP8, kind="ExternalInput"
    )

    # one f32 output row: [6 buckets * top-8 cand | 6 acc sums]
    # (buckets/acc: [pair0 | pair1 | pair2 | j6 | j7a | j7b])
    out_o = nc.dram_tensor("out", [128, NCC + NBUCK], F32, kind="ExternalOutput")
    lc_o = nc.dram_tensor("lcT", [L - 1, B], F32, kind="ExternalOutput")

    AF = mybir.ActivationFunctionType
    ALU = mybir.AluOpType

    with tile.TileContext(nc) as tc:
        with (
            tc.tile_pool(name="weights", bufs=1) as wpool,
            tc.tile_pool(name="work", bufs=1) as spool,
            tc.tile_pool(name="fqstream", bufs=NP) as fqpool,
            tc.tile_pool(name="expbuf", bufs=2) as epool,
            tc.tile_pool(name="cospair", bufs=3, space="PSUM") as pspool,
            tc.tile_pool(name="headpsum", bufs=2, space="PSUM") as hpool,
        ):
            # ---- resident params --------------------------------------
            wdq_sb = wpool.tile([128, QTB + NKC * H], FP8)
            wo_sb = wpool.tile([128, NKC, H], FP8)
            wc1_sb = wpool.tile([128, NKC, H], FP8)
            wc2_sb = wpool.tile([128, NKC, L - 1], BF16)
            bc2_sb = wpool.tile([L - 1, 1], F32)
            ones_sb = wpool.tile([128, 1], F32)
            dummy_sb = wpool.tile([128, 1], F32)

            qtb_sb = wdq_sb[:, :QTB].bitcast(BF16)
            wd_sb = wdq_sb[:, QTB:].rearrange("p (k h) -> p k h", k=NKC, h=H)
            qt_sb = qtb_sb[:, :NKC * B].rearrange("p (k b) -> p k b", k=NKC, b=B)
            bias_sb = wpool.tile([128, 3 * NKC], F32)
            bd_sb = bias_sb[:, 0:NKC]
            bo_sb = bias_sb[:, NKC:2 * NKC]
            bc1_sb = bias_sb[:, 2 * NKC:3 * NKC]

            # warm the scalar engine's activation table (tanh/exp share a
            # table) while the weight DMAs are in flight
            nc.vector.memset(dummy_sb[:], 0.0)
            nc.scalar.activation(dummy_sb[:], dummy_sb[:], AF.Tanh)
            nc.vector.memset(ones_sb[:], 1.0)

            # one DMA instruction per tensor, in critical-path order, all
            # on the same FIFO queue; the last fq pair travels as two
            # singles so the tail computes on 512 columns
            nc.sync.dma_start(wdq_sb[:], wdq.ap())
            nc.vector.tensor_copy(bias_sb[:], qtb_sb[:, NKC * B:])
            nc.sync.dma_start(wo_sb[:], wo.ap())
            nc.sync.dma_start(wc1_sb[:], wc1.ap())
            nc.sync.dma_start(wc2_sb[:], wc2.ap())
            nc.sync.dma_start(bc2_sb[:], bc2t.ap())
            fts = []
            for jp in range(NP - 1):
                ft = fqpool.tile([128, 2, NKC, NH * CHUNK], FP8, tag="fq")
                nc.sync.dma_start(ft[:], fqt.ap()[jp])
                fts.append(ft)
            f6t = fqpool.tile([128, NKC, NH * CHUNK], FP8, tag="fq6")
            nc.sync.dma_start(f6t[:], fq6.ap())
            f7ts = []
            for ss in range(2):
                f7 = fqpool.tile([128, NKC, NH * (CHUNK // 2)], FP8, tag="fq7")
                nc.sync.dma_start(f7[:], fq7.ap()[ss])
                f7ts.append(f7)

            # ---- contrastive head: linerT (unnormalized) + norms ------
            h1_sb = spool.tile([128, NKC, B], BF16)
            pre2_sb = spool.tile([128, NKC, B], F32)
            pre2b_sb = spool.tile([128, NKC, B], BF16)
            sq_sb = spool.tile([128, NKC, B], F32)

            for mc in range(NKC):
                ps = hpool.tile([128, B], F32, tag="headps")
                for kc in range(NKC):
                    nc.tensor.matmul(
                        ps[:],
                        wd_sb[:, kc, mc * 128:(mc + 1) * 128],
                        qt_sb[:, kc, :],
                        start=(kc == 0),
                        stop=(kc == NKC - 1),
                    )
                nc.scalar.activation(
                    h1_sb[:, mc, :], ps[:], AF.Tanh,
                    bias=bd_sb[:, mc:mc + 1], scale=1.0 / W_SCALE,
                )

            for mc in range(NKC):
                ps = hpool.tile([128, B], F32, tag="headps")
                for kc in range(NKC):
                    nc.tensor.matmul(
                        ps[:],
                        wo_sb[:, kc, mc * 128:(mc + 1) * 128],
                        h1_sb[:, kc, :],
                        start=(kc == 0),
                        stop=(kc == NKC - 1),
                    )
                nc.vector.tensor_scalar(
                    pre2_sb[:, mc, :], ps[:],
                    1.0 / W_SCALE, bo_sb[:, mc:mc + 1],
                    ALU.mult, ALU.add,
                )
                nc.vector.tensor_copy(pre2b_sb[:, mc, :], pre2_sb[:, mc, :])
                nc.vector.tensor_mul(sq_sb[:, mc, :], pre2_sb[:, mc, :], pre2_sb[:, mc, :])

            # row norms of liner_q: DVE-accumulate the squares over the mc
            # slices, then one ones-matmul per partition half
            sqs_sb = spool.tile([128, B], F32)
            nc.vector.tensor_add(sqs_sb[:], sq_sb[:, 0, :], sq_sb[:, 1, :])
            for mc in range(2, NKC):
                nc.vector.tensor_add(sqs_sb[:], sqs_sb[:], sq_sb[:, mc, :])

            # ---- main stream: cos pair-tiles -> exp(+sum) -> topk -----
            # two 512-col chunks accumulate into one [128, 1024] psum
            # tile; the two partition halves alternate so the PE runs
            # them concurrently in column groups (0,0) and (0,64).  The
            # last pair is processed as two 512-col steps to shorten the
            # serial exp/max8 tail after its DMA lands.  The (tiny) cls
            # head is interleaved mc-slice-wise so its matmuls fill the
            # PE's DMA-pacing gaps and its tanh chain never blocks the
            # stream; the logits matmuls run before the final two
            # singles so the lcT DMA overlaps the tail.
            out_sb = spool.tile([128, NCC + NBUCK], F32)
            cand_sb = out_sb[:, :NCC]
            acc_sb = out_sb[:, NCC:]
            h1c_sb = spool.tile([128, NKC, B], BF16)

            def cls_slice(mc):
                ps = hpool.tile([128, B], F32, tag="headps")
                for kc in range(NKC):
                    nc.tensor.matmul(
                        ps[:],
                        wc1_sb[:, kc, mc * 128:(mc + 1) * 128],
                        qt_sb[:, kc, :],
                        start=(kc == 0),
                        stop=(kc == NKC - 1),
                    )
                nc.scalar.activation(
                    h1c_sb[:, mc, :], ps[:], AF.Tanh,
                    bias=bc1_sb[:, mc:mc + 1], scale=1.0 / W_SCALE,
                )

            # two cls slices fill the PE gap between the head finishing
            # and the first fq pair landing; the norm matmuls (gated on
            # the DVE square-sum) come after so they can't head-of-line
            # block the PE behind the slower DVE chain
            cls_slice(0)
            cls_slice(1)

            ps_n = hpool.tile([128, 1], F32, tag="headps")
            for hh in range(NH):
                nc.tensor.matmul(
                    ps_n[hh * 64:(hh + 1) * 64, :],
                    sqs_sb[:],
                    ones_sb[:],
                    start=True,
                    stop=True,
                )
            # scol = 1/(T * FQ_SCALE * ||liner_q||): the sqrt's two
            # activation-table swaps overlap the first pair's matmuls
            norm_sb = spool.tile([128, 1], F32)
            rcp_sb = spool.tile([128, 1], F32)
            scol_sb = spool.tile([128, 1], F32)
            nc.scalar.activation(norm_sb[:], ps_n[:], AF.Sqrt)
            nc.vector.reciprocal(rcp_sb[:], norm_sb[:])
            nc.vector.tensor_scalar_mul(scol_sb[:], rcp_sb[:], 1.0 / (T * FQ_SCALE))

            for jp in range(NP - 1):
                ps_c = pspool.tile([128, NH * CHUNK], F32, tag="cos")
                ft = fts[jp]
                for half in range(2):
                    for kc in range(NKC):
                        for hh in range(NH):
                            nc.tensor.matmul(
                                ps_c[hh * 64:(hh + 1) * 64,
                                     half * CHUNK:(half + 1) * CHUNK],
                                pre2b_sb[:, kc, :],
                                ft[:, half, kc, hh * CHUNK:(hh + 1) * CHUNK],
                                start=(kc == 0),
                                stop=(kc == NKC - 1),
                            )
                exp_t = epool.tile([128, NH * CHUNK], BF16, tag="exp")
                nc.scalar.activation(
                    exp_t[:],
                    ps_c[:],
                    AF.Exp,
                    scale=scol_sb[:],
                    accum_out=acc_sb[:, jp:jp + 1],
                )
                nc.vector.max(cand_sb[:, jp * NCAND:(jp + 1) * NCAND], exp_t[:])
                if jp < 2:
                    cls_slice(2 * jp + 2)
                    cls_slice(2 * jp + 3)

            # classification logits: h1c completed during the pairs
            ps_l = hpool.tile([L - 1, B], F32, tag="headps")
            for kc in range(NKC):
                nc.tensor.matmul(
                    ps_l[:],
                    wc2_sb[:, kc, :],
                    h1c_sb[:, kc, :],
                    start=(kc == 0),
                    stop=(kc == NKC - 1),
                )
            lc_sb = spool.tile([L - 1, B], F32)
            nc.vector.tensor_scalar_add(lc_sb[:], ps_l[:], bc2_sb[:])
            nc.sync.dma_start(lc_o.ap(), lc_sb[:])

            # j6 as one 512-col step, j7 as two 256-col steps: the serial
            # matmul->exp->max8 chain after the last DMA byte shrinks
            ps_p = pspool.tile([128, NH * CHUNK], F32, tag="cos")
            ps_c = ps_p[:, :CHUNK]
            for kc in range(NKC):
                for hh in range(NH):
                    nc.tensor.matmul(
                        ps_c[hh * 64:(hh + 1) * 64, :],
                        pre2b_sb[:, kc, :],
                        f6t[:, kc, hh * CHUNK:(hh + 1) * CHUNK],
                        start=(kc == 0),
                        stop=(kc == NKC - 1),
                    )
            exp_t = epool.tile([128, CHUNK], BF16, tag="exp1")
            nc.scalar.activation(
                exp_t[:],
                ps_c[:],
                AF.Exp,
                scale=scol_sb[:],
                accum_out=acc_sb[:, NP - 1:NP],
            )
            nc.vector.max(
                cand_sb[:, (NP - 1) * NCAND:NP * NCAND], exp_t[:]
            )

            HC = CHUNK // 2
            for ss in range(2):
                ps_p = pspool.tile([128, NH * CHUNK], F32, tag="cos")
                ps_c = ps_p[:, :HC]
                for kc in range(NKC):
                    for hh in range(NH):
                        nc.tensor.matmul(
                            ps_c[hh * 64:(hh + 1) * 64, :],
                            pre2b_sb[:, kc, :],
                            f7ts[ss][:, kc, hh * HC:(hh + 1) * HC],
                            start=(kc == 0),
                            stop=(kc == NKC - 1),
                        )
                exp_t = epool.tile([128, HC], BF16, tag="exp2")
                col = NP + ss
                nc.scalar.activation(
                    exp_t[:],
                    ps_c[:],
                    AF.Exp,
                    scale=scol_sb[:],
                    accum_out=acc_sb[:, col:col + 1],
                )
                nc.vector.max(
                    cand_sb[:, (NP + ss) * NCAND:(NP + ss + 1) * NCAND],
                    exp_t[:],
                )

            nc.sync.dma_start(out_o.ap(), out_sb[:])

    nc.compile()
    return nc


def _get_nc():
    if "nc" not in _cache:
        _cache["nc"] = _build_nc()
    return _cache["nc"]


def _prep_inputs(q, label_queue, feature_queue, Wd, bd, Wo, bo, Wc1, bc1, Wc2, bc2):
    """Host-side shard/layout prep.  Returns per-core input maps."""
    lq = np.asarray(label_queue).astype(np.int64)
    counts = np.bincount(lq, minlength=L)
    assert counts.shape[0] == L and np.all(counts == K // L), (
        "kernel assumes an exactly balanced label queue"
    )
    perm = np.argsort(lq, kind="stable")
    fq_sorted = np.asarray(feature_queue, dtype=np.float32)[perm]  # [K, H]

    bf16 = mybir.dt.np(BF16)
    fp8 = mybir.dt.np(FP8)

    def pk(w, cols, dt, scale=1.0):  # [H, cols] -> [128, NKC, cols]
        return np.ascontiguousarray(
            (np.asarray(w, np.float32) * scale)
            .reshape(NKC, 128, cols).transpose(1, 0, 2)
        ).astype(dt)

    def pb(b):  # [H] -> [128, NKC]
        return np.asarray(b, np.float32).reshape(NKC, 128).T

    qtb = np.concatenate(
        [
            pk(np.asarray(q, np.float32).T, B, np.float32).reshape(128, -1),
            pb(bd), pb(bo), pb(bc1),
        ],
        axis=1,
    ).astype(bf16)
    # qT+biases (as raw bytes) ride in front of Wd in one fp8 tensor
    wdq = np.concatenate(
        [
            np.ascontiguousarray(qtb).view(np.uint8).view(fp8),
            pk(Wd, H, fp8, W_SCALE).reshape(128, -1),
        ],
        axis=1,
    )
    common = {
        "wdq": np.ascontiguousarray(wdq),
        "wo": pk(Wo, H, fp8, W_SCALE),
        "wc1": pk(Wc1, H, fp8, W_SCALE),
        "wc2": pk(Wc2, L - 1, bf16),
        "bc2": np.ascontiguousarray(np.asarray(bc2, np.float32).reshape(L - 1, 1)),
    }
    in_maps = []
    for c in range(NCORES):
        shard = fq_sorted[c * KSH:(c + 1) * KSH]          # [8192, H]
        fqT = np.ascontiguousarray(shard.T)               # [H, 8192]
        # [kc, r, hh, j, cc] with col = hh*4096 + j*512 + cc
        g = (fqT * FQ_SCALE).reshape(NKC, 128, NH, NJ, CHUNK)
        # fqt[jp, r, half, kc*1024 + hh*512 + cc], j = 2*jp + half
        tiles = np.ascontiguousarray(
            g[:, :, :, :NJ - 2].reshape(NKC, 128, NH, NP - 1, 2, CHUNK)
            .transpose(3, 1, 4, 0, 2, 5)
            .reshape(NP - 1, 128, 2, NKC * NH * CHUNK).astype(fp8)
        )
        t6 = np.ascontiguousarray(
            g[:, :, :, NJ - 2].transpose(1, 0, 2, 3)
            .reshape(128, NKC * NH * CHUNK).astype(fp8)
        )
        # fq7[ss, r, kc*512 + hh*256 + cc], cols [hh*4096+7*512+ss*256+cc]
        t7 = np.ascontiguousarray(
            g[:, :, :, NJ - 1].reshape(NKC, 128, NH, 2, CHUNK // 2)
            .transpose(3, 1, 0, 2, 4)
            .reshape(2, 128, NKC * NH * (CHUNK // 2)).astype(fp8)
        )
        in_maps.append({**common, "fqt": tiles, "fq6": t6, "fq7": t7})
    return in_maps


def kernel(
    q,
    labels,
    label_queue,
    feature_queue,
    Wd,
    bd,
    Wo,
    bo,
    Wc1,
    bc1,
    Wc2,
    bc2,
):
    global last_exec_time_ns, last_results
    nc = _get_nc()
    in_maps = _prep_inputs(
        q, label_queue, feature_queue, Wd, bd, Wo, bo, Wc1, bc1, Wc2, bc2
    )

    trace = os.environ.get("BASS_KERNEL_TRACE", "0") == "1"
    if trace:
        _ensure_ntff_hook()
    try:
        res = run_bass_kernel_spmd(
            nc,
            in_maps,
            core_ids=list(range(NCORES)),
            trace=trace,
            trace_cores=[0] if trace else None,
        )
    except Exception:
        if not trace:
            raise
        res = run_bass_kernel_spmd(nc, in_maps, core_ids=list(range(NCORES)))
    last_exec_time_ns = res.exec_time_ns
    last_results = res

    labels_np = np.asarray(labels).astype(np.int64)

    # ---- tiny host-side merge (the "gather + reduce" step) -----------
    O = np.stack([np.asarray(r["out"]) for r in res.results]).astype(np.float64)
    C = O[:, :, :NCC]
    A = O[:, :, NCC:]

    # per-row candidate pool: cores x halves x (6 buckets * top-8)
    cand = np.concatenate([C[:, :B, :], C[:, B:, :]], axis=2)  # [8, 64, 96]
    cand = cand.transpose(1, 0, 2).reshape(B, -1)              # [64, 768]
    e_top = np.sort(cand, axis=1)[:, ::-1][:, :TOP_K]          # exp(p/T) desc
    # Exactness proof: every unextracted value in a bucket is <= that
    # bucket's 8th-largest (MAX8 output is sorted desc).  If all bucket
    # minima are <= the global 25th candidate, the top-25 value set is
    # provably complete.
    bucket_min = np.concatenate(
        [C[:, :B, 7::8], C[:, B:, 7::8]], axis=2
    ).transpose(1, 0, 2).reshape(B, -1)                        # [64, 96]
    assert (bucket_min.max(axis=1) <= e_top[:, TOP_K - 1] + 1e-12).all(), (
        "top-k candidate extraction cannot prove exactness for this input"
    )

    # acc columns: [pair0 | pair1 | pair2 | j6 | j7a | j7b]; label group
    # g of half hh is pair g for g < 3, else j6 + j7a + j7b (label =
    # core*8 + hh*4 + g, partition p = b + 64*hh)
    S_all = A[:, :B, :].sum(axis=(0, 2)) + A[:, B:, :].sum(axis=(0, 2))  # [64]
    lam = labels_np
    c_star, r_star = np.divmod(lam, 8)
    h_star, g_star = np.divmod(r_star, 4)
    row = np.arange(B) + 64 * h_star
    S_pos = np.where(
        g_star < 3,
        A[c_star, row, np.minimum(g_star, 2)],
        A[c_star, row, 3] + A[c_star, row, 4] + A[c_star, row, 5],
    )
    S_neg = S_all - S_pos

    loss_con = float(np.mean(np.log(e_top + S_neg[:, None]) - np.log(e_top)))

    logits = np.asarray(res.results[0]["lcT"]).astype(np.float64).T  # [64, 63]
    m = logits.max(axis=1, keepdims=True)
    lse = np.log(np.exp(logits - m).sum(axis=1, keepdims=True)) + m
    logp = logits - lse
    loss_cls = float(-np.mean(logp[np.arange(B), labels_np]))

    loss = 0.5 * loss_con + 0.5 * loss_cls
    return np.asarray(loss, dtype=np.float32)
